# revision 1
# baseline (speedup 1.0000x reference)
"""Trainium2 Bass kernel for the supervised-contrastive loss (nn_KCL_69784628626020).

Strategy (8 NeuronCores, SPMD):
  - Shard anchors (rows of q, k, y) across cores: 1024 rows/core.
  - Each core computes its [1024, 8192] slab of the score matrix
    S = q_loc @ q_full^T on the tensor engine (float32r, full rate at N>=512).
  - The per-column weight w_j = 1/count(y_j) is folded into the matmul as an
    extra K=1 rank-1 update adding TAU*ln(w_j) to the scores, so that the
    scalar engine's exp(PSUM/TAU) directly produces EW_ij = exp(S_ij/TAU)*w_j.
  - Per row i:
        A_i = sum_j  EW_ij            (diag excluded)
        B_i = sum_{y_j==y_i} EW_ij    (diag excluded)
        den_i = log(A_i - B_i)
        num_i = log(kpos_i + c_i * B_i)      # c_i = count(y_i), B*c = unweighted
        loss_i = (den_i - num_i) / (c_i - 1 + K)
    A and B each come from ONE fused DVE scalar_tensor_tensor op per tile
    (compare + multiply + row-reduce).  Diagonal exclusion is data-driven
    (host-provided global row ids compared against a column iota), so the
    program is identical across cores (SPMD-safe).
  - Class counts are computed on device: row-sums of the y-equality mask give
    count(y_i) for local rows; an AllGather assembles counts for all 8192
    columns.
  - kpos_i = sum_k exp(q_i . k_ik / TAU) via fused multiply-reduce per k.
  - Final mean: per-core partial sum via a ones-matmul partition reduction;
    host adds the 8 partials (the unshard step).
"""

import numpy as np
from contextlib import ExitStack

import concourse.bass as bass
import concourse.bacc as bacc
import concourse.tile as tile
from concourse import mybir
from concourse.bass_utils import run_bass_kernel_spmd
import ml_dtypes

F32 = mybir.dt.float32
F32R = mybir.dt.float32r
F16 = mybir.dt.float16
BF16 = mybir.dt.bfloat16

TAU = 0.07
NCORES = 8


class Cfg:
    def __init__(self, N=8192, D=512, KP=8, TW=1024):
        self.N = N            # total rows (anchors)
        self.D = D            # feature dim
        self.KP = KP          # external positives per anchor
        self.TW = TW          # column tile width
        self.NL = N // NCORES     # rows per core
        self.NB = self.NL // 128  # row blocks per core
        self.NS = N // TW         # column tiles
        self.KC = D // 128        # contraction chunks
        assert self.NL % 128 == 0 and N % TW == 0 and D % 128 == 0
        assert TW % 512 == 0
        self.NCH = TW // 512      # 512-wide matmul chunks per column tile


# Engine selection knobs (tuned from traces).
STT1_ENGINES = None  # set in build_bass
STT2_ENGINES = None
KPATH_ENGINES = None


def build_bass(cfg: Cfg, stt1_eng="vector", stt2_eng="vector", k_eng="vector"):
    N, D, KP, TW = cfg.N, cfg.D, cfg.KP, cfg.TW
    NL, NB, NS, KC, NCH = cfg.NL, cfg.NB, cfg.NS, cfg.KC, cfg.NCH

    nc = bacc.Bacc("TRN2", target_bir_lowering=False, debug=False,
                   num_devices=NCORES)

    # ---- kernel I/O -------------------------------------------------------
    qT_d = nc.dram_tensor("qT", [KC, 128, N], F32R, kind="ExternalInput")
    qTl_d = nc.dram_tensor("qTl", [KC, 128, NL], F32R, kind="ExternalInput")
    kr_d = nc.dram_tensor("kr", [NB, 128, KP * D], BF16, kind="ExternalInput")
    qr_d = nc.dram_tensor("qr", [NB, 128, D], F32, kind="ExternalInput")
    ybc_d = nc.dram_tensor("ybc", [128, N], F16, kind="ExternalInput")
    yrow_d = nc.dram_tensor("yrow", [128, NB], F32, kind="ExternalInput")
    colid_d = nc.dram_tensor("colid", [128, TW], F16, kind="ExternalInput")
    rowadj_d = nc.dram_tensor("rowadj", [128, NB * NS], F32, kind="ExternalInput")
    out_d = nc.dram_tensor("out", [1, 1], F32, kind="ExternalOutput")

    eng = {"vector": nc.vector, "gpsimd": nc.gpsimd}
    stt1e = eng[stt1_eng]
    stt2e = eng[stt2_eng]
    ke = eng[k_eng]

    with tile.TileContext(nc) as tc, ExitStack() as ctx:
        const = ctx.enter_context(tc.tile_pool(name="const", bufs=1))
        rh_pool = ctx.enter_context(tc.tile_pool(name="rh", bufs=8))
        psum_pool = ctx.enter_context(tc.tile_pool(name="ps", bufs=3, space="PSUM"))
        ew_pool = ctx.enter_context(tc.tile_pool(name="ew", bufs=3))
        t1_pool = ctx.enter_context(tc.tile_pool(name="t1", bufs=3))
        t2_pool = ctx.enter_context(tc.tile_pool(name="t2", bufs=2))
        k_pool = ctx.enter_context(tc.tile_pool(name="kp", bufs=2))
        q_pool = ctx.enter_context(tc.tile_pool(name="qp", bufs=2))
        dram = ctx.enter_context(tc.tile_pool(name="dram", bufs=1, space="DRAM"))

        # ---- resident constants ------------------------------------------
        qtl = [const.tile([128, NL], F32R, tag=f"qtl{c}", name=f"qtl{c}") for c in range(KC)]
        for c in range(KC):
            nc.sync.dma_start(qtl[c][:, :], qTl_d[c, :, :])
        ybc = const.tile([128, N], F16, tag="ybc")
        nc.sync.dma_start(ybc[:, :], ybc_d[:, :])
        colid = const.tile([128, TW], F16, tag="colid")
        nc.sync.dma_start(colid[:, :], colid_d[:, :])
        yrow = const.tile([128, NB], F32, tag="yrow")
        nc.sync.dma_start(yrow[:, :], yrow_d[:, :])
        rowadj = const.tile([128, NB * NS], F32, tag="rowadj")
        nc.sync.dma_start(rowadj[:, :], rowadj_d[:, :])

        ones_k1 = const.tile([1, 128], F32R, tag="ones_k1")
        nc.vector.memset(ones_k1[:, :].bitcast(F32), 1.0)
        ones_col = const.tile([128, 1], F32, tag="ones_col")
        nc.vector.memset(ones_col[:, :], 1.0)

        # accumulator slots
        aslt = const.tile([128, NB * NS], F32, tag="aslt")
        bslt = const.tile([128, NB * NS], F32, tag="bslt")
        kss = const.tile([128, NB * KP], F32, tag="kss")
        kpos = const.tile([128, NB], F32, tag="kpos")
        cloc = const.tile([128, NB], F32, tag="cloc")
        losscol = const.tile([128, NB], F32, tag="losscol")

        # ---- phase W: class counts + lw ----------------------------------
        cnt_scr = const.tile([128, N], F16, tag="cnt_scr")
        for b in range(NB):
            nc.vector.tensor_scalar(
                cnt_scr[:, :], ybc[:, :], yrow[:, b:b + 1], None,
                op0=mybir.AluOpType.is_equal,
                op1=mybir.AluOpType.add,
                accum_out=cloc[:, b:b + 1])

        cpart = dram.tile([1, NL], F32)
        call = dram.tile([NCORES, NL], F32, addr_space="Shared")
        # cpart[0, b*128+p] = cloc[p, b]
        nc.sync.dma_start(
            cpart[:, :].rearrange("o (b p) -> p (o b)", b=NB, p=128),
            cloc[:, :])
        nc.gpsimd.collective_compute(
            "AllGather", mybir.AluOpType.bypass,
            ins=[cpart[:, :].opt()],
            outs=[call[:, :].opt()],
            replica_groups=[list(range(NCORES))],
        )
        # counts for all N columns -> SBUF [128, N/128] (global row-major)
        NF = N // 128
        csb = const.tile([128, NF], F32, tag="csb")
        nc.sync.dma_start(
            csb[:, :],
            call[:, :].rearrange("r l -> (r l)").rearrange("(p f) -> p f", p=128, f=NF))
        lnc = const.tile([128, NF], F32, tag="lnc")
        nc.scalar.activation(lnc[:, :], csb[:, :], mybir.ActivationFunctionType.Ln)
        lwsb = const.tile([128, NF], F32R, tag="lwsb")
        nc.vector.tensor_scalar_mul(lwsb[:, :], lnc[:, :], -TAU)
        lw_d = dram.tile([1, N], F32R)
        nc.sync.dma_start(
            lw_d[:, :].rearrange("o (p f) -> p (o f)", p=128, f=NF),
            lwsb[:, :])
        lwrow = const.tile([1, N], F32R, tag="lwrow")
        nc.sync.dma_start(lwrow[:, :], lw_d[:, :])

        # ---- main loop: score slab ---------------------------------------
        for s in range(NS):
            rhs = [rh_pool.tile([128, TW], F32R, tag="rh", name=f"rhs{s}_{c2}") for c2 in range(KC)]
            for c in range(KC):
                nc.sync.dma_start(rhs[c][:, :], qT_d[c, :, s * TW:(s + 1) * TW])
            for b in range(NB):
                ps = psum_pool.tile([128, TW], F32)
                for nch in range(NCH):
                    o = ps[:, nch * 512:(nch + 1) * 512]
                    for c in range(KC):
                        nc.tensor.matmul(
                            o,
                            qtl[c][:, b * 128:(b + 1) * 128],
                            rhs[c][:, nch * 512:(nch + 1) * 512],
                            start=(c == 0), stop=False)
                    nc.tensor.matmul(
                        o,
                        ones_k1[0:1, :],
                        lwrow[0:1, s * TW + nch * 512: s * TW + (nch + 1) * 512],
                        start=False, stop=True)
                ew = ew_pool.tile([128, TW], F32)
                nc.scalar.activation(ew[:, :], ps[:, :],
                                     mybir.ActivationFunctionType.Exp,
                                     scale=float(1.0 / TAU))
                # A: zero the diagonal, row-sum everything
                t1 = t1_pool.tile([128, TW], F32)
                stt1e.scalar_tensor_tensor(
                    t1[:, :], colid[:, :], rowadj[:, (b * NS + s):(b * NS + s) + 1],
                    ew[:, :],
                    op0=mybir.AluOpType.not_equal, op1=mybir.AluOpType.mult,
                    accum_out=aslt[:, (b * NS + s):(b * NS + s) + 1])
                # B: same-class row-sum (diag already zeroed in t1)
                t2 = t2_pool.tile([128, TW], F16)
                stt2e.scalar_tensor_tensor(
                    t2[:, :], ybc[:, s * TW:(s + 1) * TW], yrow[:, b:b + 1],
                    t1[:, :],
                    op0=mybir.AluOpType.is_equal, op1=mybir.AluOpType.mult,
                    accum_out=bslt[:, (b * NS + s):(b * NS + s) + 1])

        # ---- k-path: kpos = sum_k exp(q.k/TAU) ---------------------------
        for b in range(NB):
            kt = k_pool.tile([128, KP * D], BF16, tag="kt")
            nc.sync.dma_start(kt[:, :], kr_d[b, :, :])
            qt = q_pool.tile([128, D], F32, tag="qt")
            nc.sync.dma_start(qt[:, :], qr_d[b, :, :])
            for kk in range(KP):
                kscr = q_pool.tile([128, D], BF16, tag="kscr")
                ke.scalar_tensor_tensor(
                    kscr[:, :], kt[:, kk * D:(kk + 1) * D], 1.0,
                    qt[:, :],
                    op0=mybir.AluOpType.mult, op1=mybir.AluOpType.mult,
                    accum_out=kss[:, b * KP + kk: b * KP + kk + 1])
            ksse = const.tile([128, KP], F32, tag=f"ksse{b}")
            nc.scalar.activation(
                ksse[:, :],
                kss[:, b * KP:(b + 1) * KP],
                mybir.ActivationFunctionType.Exp, scale=float(1.0 / TAU),
                accum_out=kpos[:, b:b + 1])

        # ---- finalize per row block --------------------------------------
        fin = const.tile([128, 6 * NB], F32, tag="fin")
        for b in range(NB):
            acol = fin[:, 6 * b + 0: 6 * b + 1]
            bcol = fin[:, 6 * b + 1: 6 * b + 2]
            nc.vector.tensor_reduce(acol, aslt[:, b * NS:(b + 1) * NS],
                                    mybir.AxisListType.X, mybir.AluOpType.add)
            nc.vector.tensor_reduce(bcol, bslt[:, b * NS:(b + 1) * NS],
                                    mybir.AxisListType.X, mybir.AluOpType.add)
            den_in = fin[:, 6 * b + 2: 6 * b + 3]
            nc.vector.tensor_sub(den_in, acol, bcol)
            num_in = fin[:, 6 * b + 3: 6 * b + 4]
            # num_in = kpos + cloc * B
            nc.vector.scalar_tensor_tensor(
                num_in, bcol, cloc[:, b:b + 1], kpos[:, b:b + 1],
                op0=mybir.AluOpType.mult, op1=mybir.AluOpType.add)
            den_l = fin[:, 6 * b + 4: 6 * b + 5]
            nc.scalar.activation(den_l, den_in, mybir.ActivationFunctionType.Ln)
            num_l = fin[:, 6 * b + 5: 6 * b + 6]
            nc.scalar.activation(num_l, num_in, mybir.ActivationFunctionType.Ln)
        # losscol[:, b] = (den_l - num_l) / (cloc - 1 + KP)
        dinv_t = const.tile([128, NB], F32, tag="dinv")
        tmp_t = const.tile([128, NB], F32, tag="tmpd")
        nc.vector.tensor_scalar_add(tmp_t[:, :], cloc[:, :], float(KP - 1))
        nc.vector.reciprocal(dinv_t[:, :], tmp_t[:, :])
        for b in range(NB):
            den_l = fin[:, 6 * b + 4: 6 * b + 5]
            num_l = fin[:, 6 * b + 5: 6 * b + 6]
            diff = fin[:, 6 * b + 2: 6 * b + 3]  # overwrite den_in
            nc.vector.tensor_sub(diff, den_l, num_l)
            nc.vector.tensor_mul(losscol[:, b:b + 1], diff, dinv_t[:, b:b + 1])

        # ---- reduce to a single partial ----------------------------------
        lsum = const.tile([128, 1], F32, tag="lsum")
        nc.vector.tensor_reduce(lsum[:, :], losscol[:, :],
                                mybir.AxisListType.X, mybir.AluOpType.add)
        psf = psum_pool.tile([128, 512], F32, bufs=1)
        nc.tensor.matmul(psf[0:1, 0:1], lsum[:, :],
                         ones_col[:, :], start=True, stop=True)
        outsb = const.tile([1, 1], F32, tag="outsb")
        nc.scalar.copy(outsb[0:1, 0:1], psf[0:1, 0:1])
        nc.sync.dma_start(out_d[:, :], outsb[0:1, 0:1])

    nc.compile()
    return nc


# ---------------------------------------------------------------------------
# host-side marshalling
# ---------------------------------------------------------------------------

def make_inputs(q, k, y, cfg: Cfg):
    """Build the per-core input maps (pure layout/replication marshalling)."""
    N, D, KP, TW = cfg.N, cfg.D, cfg.KP, cfg.TW
    NL, NB, NS, KC = cfg.NL, cfg.NB, cfg.NS, cfg.KC
    q = np.asarray(q, dtype=np.float32)
    k = np.asarray(k, dtype=np.float32)
    y = np.asarray(y)

    qT = np.ascontiguousarray(q.T).reshape(KC, 128, N)
    ybc = np.broadcast_to(y.astype(np.float16)[None, :], (128, N)).copy()
    colid = np.broadcast_to(np.arange(TW, dtype=np.float16)[None, :], (128, TW)).copy()

    in_maps = []
    for r in range(NCORES):
        rows = slice(r * NL, (r + 1) * NL)
        qTl = np.ascontiguousarray(q[rows].T).reshape(KC, 128, NL)
        kr = np.ascontiguousarray(k[rows].reshape(NB, 128, KP * D)).astype(ml_dtypes.bfloat16)
        qr = np.ascontiguousarray(q[rows].reshape(NB, 128, D))
        yrow = np.ascontiguousarray(y[rows].astype(np.float32).reshape(NB, 128).T)
        # rowadj[p, b*NS+s] = global_row - s*TW
        p = np.arange(128, dtype=np.float32)
        badx = np.arange(NB, dtype=np.float32)
        sadx = np.arange(NS, dtype=np.float32)
        grow = r * NL + badx[:, None, None] * 128 + p[None, :, None]  # [NB,128,1]
        rowadj = (grow - sadx[None, None, :] * TW)                   # [NB,128,NS]
        rowadj = np.ascontiguousarray(rowadj.transpose(1, 0, 2).reshape(128, NB * NS),
                                      dtype=np.float32)
        in_maps.append({
            "qT": qT, "qTl": qTl, "kr": kr, "qr": qr,
            "ybc": ybc, "yrow": yrow, "colid": colid, "rowadj": rowadj,
        })
    return in_maps


_CACHE = {}


def _get_nc(cfg_key):
    if cfg_key not in _CACHE:
        cfg = Cfg()
        _CACHE[cfg_key] = (cfg, build_bass(cfg))
    return _CACHE[cfg_key]


def kernel(q, k, y, trace=False):
    cfg, nc = _get_nc("full")
    in_maps = make_inputs(q, k, y, cfg)
    res = run_bass_kernel_spmd(nc, in_maps, core_ids=list(range(NCORES)),
                               trace=trace)
    total = np.sum([res.results[r]["out"][0, 0] for r in range(NCORES)],
                   dtype=np.float64)
    out = np.asarray(total / cfg.N, dtype=np.float32)
    if trace:
        kernel.last_results = res
    return out



# revision 3
# speedup vs baseline: 1.9269x; 1.9269x over previous
"""Trainium2 Bass kernel for the supervised-contrastive loss (nn_KCL_69784628626020).

Strategy (8 NeuronCores, SPMD):
  - Shard anchors (rows of q, k, y) across cores: 1024 rows/core.
  - Each core computes its [1024, 8192] slab of the score matrix
    S = q_loc @ q_full^T on the tensor engine.  In fp8 mode the q operands
    are prescaled by 16 and cast to e4m3, and pairs of 128-deep contraction
    chunks run in DoubleRow perf mode (2 cols/cycle).
  - Per-column class weights w_j = 1/count(y_j) are computed on the host
    (np.bincount marshalling) and shipped as an fp16 row broadcast; no
    collective and no in-PSUM rank-1 update is needed.
  - The column space of each core is ROTATED by r*NL so that the diagonal
    block always lands in column-tile s=0.  Tile 0 uses special label /
    weight inputs (ybc0/wbc0) with the diagonal position poisoned
    (label 0 with labels shifted +1, weight 0), so the diagonal is excluded
    from both reductions with zero extra ops and no large-term cancellation.
  - Per (s,b) tile: matmuls -> PSUM, scalar-engine exp -> bf16 ew tile,
    then two DVE scalar_tensor_tensor reduces (4x 16-bit mode):
        A_w[i] += sum_j w_j * ew_ij      (weighted, all classes)
        B[i]   += sum_{y_j==y_i} ew_ij   (unweighted, same class, no diag)
    Finalize per row:
        den_i = A_w_i - w_i * B_i        (same-class part cancels exactly)
        num_i = kpos_i + B_i
        loss_i = (ln den_i - ln num_i) / (c_i - 1 + K)
  - kpos_i = sum_k exp(q_i . k_ik / TAU) via gpsimd multiply-reduce per k
    (fp16 inputs), exp+accum on the scalar engine.
  - Final mean: per-core partial sum via a ones-matmul partition reduction;
    host adds the 8 partials (the unshard step).
"""

import numpy as np
from contextlib import ExitStack

import concourse.bass as bass
import concourse.bacc as bacc
import concourse.tile as tile
from concourse import mybir
from concourse.bass_utils import run_bass_kernel_spmd
import ml_dtypes

F32 = mybir.dt.float32
F32R = mybir.dt.float32r
F16 = mybir.dt.float16
BF16 = mybir.dt.bfloat16
FP8 = mybir.dt.float8e4

TAU = 0.07
NCORES = 8
NUM_CLASSES = 1000


class Cfg:
    def __init__(self, N=8192, D=512, KP=8, TW=1024, mode="fp8dr"):
        self.N = N            # total rows (anchors)
        self.D = D            # feature dim
        self.KP = KP          # external positives per anchor
        self.TW = TW          # column tile width
        self.mode = mode      # "fp8dr" | "bf16" | "f32r"
        self.NL = N // NCORES     # rows per core
        self.NB = self.NL // 128  # row blocks per core
        self.NS = N // TW         # column tiles
        self.KC = D // 128        # contraction chunks
        assert self.NL % 128 == 0 and N % TW == 0 and D % 128 == 0
        assert TW % 512 == 0 and TW <= 128 * N // self.NL
        assert self.NB <= self.NS
        self.NCH = TW // 512      # 512-wide matmul chunks per column tile
        # prescale keeps fp8 q elements in the e4m3 normal range
        self.prescale = 16.0 if mode == "fp8dr" else 1.0


def build_bass(cfg: Cfg, k_eng="vector"):
    N, D, KP, TW = cfg.N, cfg.D, cfg.KP, cfg.TW
    NL, NB, NS, KC, NCH = cfg.NL, cfg.NB, cfg.NS, cfg.KC, cfg.NCH
    mode = cfg.mode
    qdt = {"fp8dr": FP8, "bf16": BF16, "f32r": F32R}[mode]
    exp_scale = float(1.0 / (cfg.prescale * cfg.prescale * TAU))

    nc = bacc.Bacc("TRN2", target_bir_lowering=False, debug=False,
                   num_devices=NCORES)

    # ---- kernel I/O -------------------------------------------------------
    qlhs_d = nc.dram_tensor("qlhs", [128, KC, NL], qdt, kind="ExternalInput")
    qrhs_d = nc.dram_tensor("qrhs", [128, KC, N], qdt, kind="ExternalInput")
    ybc_d = nc.dram_tensor("ybc", [128, N], F16, kind="ExternalInput")
    wbc_d = nc.dram_tensor("wbc", [128, N], F16, kind="ExternalInput")
    ybc0_d = nc.dram_tensor("ybc0", [128, NB * TW], F16, kind="ExternalInput")
    wbc0_d = nc.dram_tensor("wbc0", [128, NB * TW], F16, kind="ExternalInput")
    yrow_d = nc.dram_tensor("yrow", [128, NB], F32, kind="ExternalInput")
    wrow_d = nc.dram_tensor("wrow", [128, NB], F32, kind="ExternalInput")
    dinv_d = nc.dram_tensor("dinv", [128, NB], F32, kind="ExternalInput")
    kr_d = nc.dram_tensor("kr", [NB, 128, KP * D], F16, kind="ExternalInput")
    qr_d = nc.dram_tensor("qr", [NB, 128, D], F16, kind="ExternalInput")
    out_d = nc.dram_tensor("out", [1, 1], F32, kind="ExternalOutput")

    eng = {"vector": nc.vector, "gpsimd": nc.gpsimd}
    ke = eng[k_eng]

    with tile.TileContext(nc) as tc, ExitStack() as ctx:
        const = ctx.enter_context(tc.tile_pool(name="const", bufs=1))
        rh_pool = ctx.enter_context(tc.tile_pool(name="rh", bufs=3))
        psum_pool = ctx.enter_context(tc.tile_pool(name="ps", bufs=3, space="PSUM"))
        ew_pool = ctx.enter_context(tc.tile_pool(name="ew", bufs=4))
        dump_pool = ctx.enter_context(tc.tile_pool(name="dmp", bufs=3))
        k_pool = ctx.enter_context(tc.tile_pool(name="kp", bufs=2))
        q_pool = ctx.enter_context(tc.tile_pool(name="qp", bufs=2))
        kscr_pool = ctx.enter_context(tc.tile_pool(name="ks", bufs=2))

        # ---- resident inputs (priority order for DMA) --------------------
        qlhs = const.tile([128, KC, NL], qdt, tag="qlhs")
        nc.sync.dma_start(qlhs[:, :, :], qlhs_d[:, :, :])
        ybc0 = const.tile([128, NB * TW], F16, tag="ybc0")
        nc.sync.dma_start(ybc0[:, :], ybc0_d[:, :])
        wbc0 = const.tile([128, NB * TW], F16, tag="wbc0")
        nc.sync.dma_start(wbc0[:, :], wbc0_d[:, :])
        yrow = const.tile([128, NB], F32, tag="yrow")
        nc.sync.dma_start(yrow[:, :], yrow_d[:, :])
        wrow = const.tile([128, NB], F32, tag="wrow")
        nc.sync.dma_start(wrow[:, :], wrow_d[:, :])
        dinv = const.tile([128, NB], F32, tag="dinv")
        nc.sync.dma_start(dinv[:, :], dinv_d[:, :])
        ybc = const.tile([128, N], F16, tag="ybc")
        nc.sync.dma_start(ybc[:, :], ybc_d[:, :])
        wbc = const.tile([128, N], F16, tag="wbc")
        nc.sync.dma_start(wbc[:, :], wbc_d[:, :])

        ones_col = const.tile([128, 1], F32, tag="ones_col")
        nc.vector.memset(ones_col[:, :], 1.0)

        # accumulator slots
        aslt = const.tile([128, NB * NS], F32, tag="aslt")
        bslt = const.tile([128, NB * NS], F32, tag="bslt")
        kss = const.tile([128, NB * KP], F32, tag="kss")
        kpos = const.tile([128, NB], F32, tag="kpos")

        # ---- main loop: score slab ---------------------------------------
        for s in range(NS):
            rhs = rh_pool.tile([128, KC, TW], qdt, tag="rh", name=f"rhs{s}")
            nc.sync.dma_start(rhs[:, :, :], qrhs_d[:, :, s * TW:(s + 1) * TW])

            # interleave k-path block s with the main loop
            if s < NB:
                kt = k_pool.tile([128, KP * D], F16, tag="kt")
                nc.sync.dma_start(kt[:, :], kr_d[s, :, :])
                qt = q_pool.tile([128, D], F16, tag="qt")
                nc.sync.dma_start(qt[:, :], qr_d[s, :, :])
                for kk in range(KP):
                    kscr = kscr_pool.tile([128, D], F16, tag="kscr")
                    ke.scalar_tensor_tensor(
                        kscr[:, :], kt[:, kk * D:(kk + 1) * D], 1.0,
                        qt[:, :],
                        op0=mybir.AluOpType.mult, op1=mybir.AluOpType.mult,
                        accum_out=kss[:, s * KP + kk: s * KP + kk + 1])
                ksse = const.tile([128, KP], F32, tag=f"ksse{s}")
                nc.scalar.activation(
                    ksse[:, :],
                    kss[:, s * KP:(s + 1) * KP],
                    mybir.ActivationFunctionType.Exp, scale=float(1.0 / TAU),
                    accum_out=kpos[:, s:s + 1])

            for b in range(NB):
                ps = psum_pool.tile([128, TW], F32)
                for nch in range(NCH):
                    o = ps[:, nch * 512:(nch + 1) * 512]
                    if mode == "fp8dr":
                        for c2 in range(KC // 2):
                            nc.tensor.matmul(
                                o,
                                qlhs[:, 2 * c2:2 * c2 + 2, b * 128:(b + 1) * 128],
                                rhs[:, 2 * c2:2 * c2 + 2, nch * 512:(nch + 1) * 512],
                                start=(c2 == 0), stop=(c2 == KC // 2 - 1),
                                perf_mode=mybir.MatmulPerfMode.DoubleRow)
                    else:
                        for c in range(KC):
                            nc.tensor.matmul(
                                o,
                                qlhs[:, c, b * 128:(b + 1) * 128],
                                rhs[:, c, nch * 512:(nch + 1) * 512],
                                start=(c == 0), stop=(c == KC - 1))
                ew = ew_pool.tile([128, TW], BF16)
                nc.scalar.activation(ew[:, :], ps[:, :],
                                     mybir.ActivationFunctionType.Exp,
                                     scale=exp_scale)
                if s == 0:
                    y_in = ybc0[:, b * TW:(b + 1) * TW]
                    w_in = wbc0[:, b * TW:(b + 1) * TW]
                else:
                    y_in = ybc[:, s * TW:(s + 1) * TW]
                    w_in = wbc[:, s * TW:(s + 1) * TW]
                # A_w: weighted full-row sum (diag excluded at s=0 by wbc0=0)
                d1 = dump_pool.tile([128, TW], BF16)
                nc.vector.scalar_tensor_tensor(
                    d1[:, :], w_in, 1.0, ew[:, :],
                    op0=mybir.AluOpType.mult, op1=mybir.AluOpType.mult,
                    accum_out=aslt[:, (b * NS + s):(b * NS + s) + 1])
                # B: same-class row-sum (diag excluded at s=0 by ybc0=0)
                d2 = dump_pool.tile([128, TW], BF16)
                nc.vector.scalar_tensor_tensor(
                    d2[:, :], y_in, yrow[:, b:b + 1], ew[:, :],
                    op0=mybir.AluOpType.is_equal, op1=mybir.AluOpType.mult,
                    accum_out=bslt[:, (b * NS + s):(b * NS + s) + 1])

        # ---- finalize (vectorized over [128, NB]) ------------------------
        acolM = const.tile([128, NB], F32, tag="acolM")
        bcolM = const.tile([128, NB], F32, tag="bcolM")
        for b in range(NB):
            nc.vector.tensor_reduce(acolM[:, b:b + 1], aslt[:, b * NS:(b + 1) * NS],
                                    mybir.AxisListType.X, mybir.AluOpType.add)
            nc.vector.tensor_reduce(bcolM[:, b:b + 1], bslt[:, b * NS:(b + 1) * NS],
                                    mybir.AxisListType.X, mybir.AluOpType.add)
        numM = const.tile([128, NB], F32, tag="numM")
        nc.vector.tensor_add(numM[:, :], bcolM[:, :], kpos[:, :])
        wbM = const.tile([128, NB], F32, tag="wbM")
        nc.vector.tensor_mul(wbM[:, :], bcolM[:, :], wrow[:, :])
        denM = const.tile([128, NB], F32, tag="denM")
        nc.vector.tensor_sub(denM[:, :], acolM[:, :], wbM[:, :])
        denL = const.tile([128, NB], F32, tag="denL")
        nc.scalar.activation(denL[:, :], denM[:, :], mybir.ActivationFunctionType.Ln)
        numL = const.tile([128, NB], F32, tag="numL")
        nc.scalar.activation(numL[:, :], numM[:, :], mybir.ActivationFunctionType.Ln)
        diffM = const.tile([128, NB], F32, tag="diffM")
        nc.vector.tensor_sub(diffM[:, :], denL[:, :], numL[:, :])
        losscol = const.tile([128, NB], F32, tag="losscol")
        nc.vector.tensor_mul(losscol[:, :], diffM[:, :], dinv[:, :])

        # ---- reduce to a single partial ----------------------------------
        lsum = const.tile([128, 1], F32, tag="lsum")
        nc.vector.tensor_reduce(lsum[:, :], losscol[:, :],
                                mybir.AxisListType.X, mybir.AluOpType.add)
        psf = psum_pool.tile([128, 512], F32, bufs=1)
        nc.tensor.matmul(psf[0:1, 0:1], lsum[:, :],
                         ones_col[:, :], start=True, stop=True)
        outsb = const.tile([1, 1], F32, tag="outsb")
        nc.scalar.copy(outsb[0:1, 0:1], psf[0:1, 0:1])
        nc.sync.dma_start(out_d[:, :], outsb[0:1, 0:1])

    nc.compile()
    return nc


# ---------------------------------------------------------------------------
# host-side marshalling
# ---------------------------------------------------------------------------

def make_inputs(q, k, y, cfg: Cfg):
    """Build the per-core input maps (layout/replication marshalling)."""
    N, D, KP, TW = cfg.N, cfg.D, cfg.KP, cfg.TW
    NL, NB, NS, KC = cfg.NL, cfg.NB, cfg.NS, cfg.KC
    q = np.asarray(q, dtype=np.float32)
    k = np.asarray(k, dtype=np.float32)
    y = np.asarray(y).astype(np.int64)

    qdt_np = {"fp8dr": ml_dtypes.float8_e4m3fn,
              "bf16": ml_dtypes.bfloat16,
              "f32r": np.float32}[cfg.mode]
    qs = (q * cfg.prescale).astype(qdt_np)          # [N, D] quantized

    counts = np.bincount(y, minlength=NUM_CLASSES)
    w16 = (1.0 / counts[y].astype(np.float64)).astype(np.float16)  # [N]
    ylab = (y + 1).astype(np.float16)                              # labels 1..C

    pidx = np.arange(128)
    in_maps = []
    for r in range(NCORES):
        rows = slice(r * NL, (r + 1) * NL)
        rot = (np.arange(N) + r * NL) % N

        # lhsT chunks: qlhs[p, c, i] = qs[r*NL+i, c*128+p]
        qlhs = np.ascontiguousarray(
            qs[rows].T.reshape(KC, 128, NL).transpose(1, 0, 2))
        # rhs chunks, rotated: qrhs[p, c, j] = qs[rot(j), c*128+p]
        qrhs = np.ascontiguousarray(
            qs[rot].T.reshape(KC, 128, N).transpose(1, 0, 2))

        ybc = np.broadcast_to(ylab[rot][None, :], (128, N)).copy()
        wbc = np.broadcast_to(w16[rot][None, :], (128, N)).copy()

        # tile-0 variants with the diagonal position poisoned per row block
        ybc0 = np.broadcast_to(ylab[rot[:TW]][None, None, :], (128, NB, TW)).copy()
        wbc0 = np.broadcast_to(w16[rot[:TW]][None, None, :], (128, NB, TW)).copy()
        bidx = np.arange(NB)
        ybc0[pidx[:, None], bidx[None, :], bidx[None, :] * 128 + pidx[:, None]] = 0.0
        wbc0[pidx[:, None], bidx[None, :], bidx[None, :] * 128 + pidx[:, None]] = 0.0

        yloc = y[rows]
        yrow = np.ascontiguousarray(
            (yloc + 1).astype(np.float32).reshape(NB, 128).T)
        wrow = np.ascontiguousarray(
            w16[rows].astype(np.float32).reshape(NB, 128).T)
        dinv = np.ascontiguousarray(
            (1.0 / (counts[yloc] - 1 + KP)).astype(np.float32).reshape(NB, 128).T)

        kr = np.ascontiguousarray(
            k[rows].reshape(NB, 128, KP * D)).astype(np.float16)
        qr = np.ascontiguousarray(
            q[rows].reshape(NB, 128, D)).astype(np.float16)

        in_maps.append({
            "qlhs": qlhs, "qrhs": qrhs,
            "ybc": ybc, "wbc": wbc,
            "ybc0": ybc0.reshape(128, NB * TW),
            "wbc0": wbc0.reshape(128, NB * TW),
            "yrow": yrow, "wrow": wrow, "dinv": dinv,
            "kr": kr, "qr": qr,
        })
    return in_maps


_CACHE = {}


def _get_nc(mode="fp8dr"):
    if mode not in _CACHE:
        cfg = Cfg(mode=mode)
        _CACHE[mode] = (cfg, build_bass(cfg))
    return _CACHE[mode]


def kernel(q, k, y, trace=False, mode="fp8dr"):
    cfg, nc = _get_nc(mode)
    in_maps = make_inputs(q, k, y, cfg)
    res = run_bass_kernel_spmd(nc, in_maps, core_ids=list(range(NCORES)),
                               trace=trace)
    total = np.sum([res.results[r]["out"][0, 0] for r in range(NCORES)],
                   dtype=np.float64)
    out = np.asarray(total / cfg.N, dtype=np.float32)
    if trace:
        kernel.last_results = res
    return out


# revision 7
# speedup vs baseline: 2.2941x; 1.1906x over previous
"""Trainium2 Bass kernel for the supervised-contrastive loss (nn_KCL_69784628626020).

Strategy (8 NeuronCores, SPMD):
  - Shard anchors (rows of q, k, y) across cores: 1024 rows/core.
  - Each core computes its [1024, 8192] slab of the score matrix
    S = q_loc @ q_full^T on the tensor engine.  In fp8 mode the q operands
    are prescaled by 16 and cast to e4m3, and pairs of 128-deep contraction
    chunks run in DoubleRow perf mode (2 cols/cycle).
  - Per-column class weights w_j = 1/count(y_j) come from a host-side
    bincount (input marshalling); their logs are folded into the scores as
    a rank-1 (ones x lw) matmul into PSUM, so the scalar engine's
    exp(scale*PSUM) directly produces EW_ij = exp(s_ij/TAU) * w_j and its
    free accum_out gives the weighted row sum A_i = sum_j EW_ij per tile.
  - The column space of each core is ROTATED by r*NL so the diagonal block
    always lands in column-tile s=0.  There, the 128 diagonal scores per
    row block are zeroed IN PSUM by one small [128,128] DVE op (compare
    column-iota vs partition-iota, multiply), making the diagonal's
    post-exp contribution exactly 1.0 in every reduction.
  - Per (s,b) tile after the exp, ONE DVE scalar_tensor_tensor computes
        B_i += sum_{y_j==y_i} EW_ij     (same-class weighted sum)
  - Finalize per row (c = class count, w16 = fp16(1/c)):
        den_i = A_i - B_i               (diagonal 1s cancel exactly)
        num_i = kpos_i + c_i * (B_i - 1)
        loss_i = (ln den_i - ln num_i) / (c_i - 1 + K)
  - kpos_i = sum_k exp(q_i . k_ik / TAU) via DVE multiply-reduce per k
    (fp16 inputs), exp+accum on the scalar engine.
  - Final mean: per-core partial sum via a ones-matmul partition reduction;
    host adds the 8 partials (the unshard step).
"""

import numpy as np
from contextlib import ExitStack

import concourse.bass as bass
import concourse.bacc as bacc
import concourse.tile as tile
from concourse import mybir
from concourse.bass_utils import run_bass_kernel_spmd
import ml_dtypes

F32 = mybir.dt.float32
F32R = mybir.dt.float32r
F16 = mybir.dt.float16
BF16 = mybir.dt.bfloat16
FP8 = mybir.dt.float8e4

TAU = 0.07
NCORES = 8
NUM_CLASSES = 1000


class Cfg:
    def __init__(self, N=8192, D=512, KP=8, TW=2048, mode="bf16"):
        self.N = N            # total rows (anchors)
        self.D = D            # feature dim
        self.KP = KP          # external positives per anchor
        self.TW = TW          # column tile width
        self.mode = mode      # "fp8dr" | "bf16" | "f32r"
        self.NL = N // NCORES     # rows per core
        self.NB = self.NL // 128  # row blocks per core
        self.NS = N // TW         # column tiles
        self.KC = D // 128        # contraction chunks
        assert self.NL % 128 == 0 and N % TW == 0 and D % 128 == 0
        assert TW % 512 == 0 and self.NL <= TW
        self.NCH = TW // 512      # 512-wide matmul chunks per column tile
        # prescale keeps fp8 q elements in the e4m3 normal range
        self.prescale = 16.0 if mode == "fp8dr" else 1.0


def build_bass(cfg: Cfg, k_eng="vector"):
    N, D, KP, TW = cfg.N, cfg.D, cfg.KP, cfg.TW
    NL, NB, NS, KC, NCH = cfg.NL, cfg.NB, cfg.NS, cfg.KC, cfg.NCH
    mode = cfg.mode
    qdt = {"fp8dr": FP8, "bf16": BF16, "f32r": F32R}[mode]
    exp_scale = float(1.0 / (cfg.prescale * cfg.prescale * TAU))

    nc = bacc.Bacc("TRN2", target_bir_lowering=False, debug=False,
                   num_devices=NCORES)

    # ---- kernel I/O -------------------------------------------------------
    qlhs_d = nc.dram_tensor("qlhs", [128, KC, NL], qdt, kind="ExternalInput")
    qrhs_d = nc.dram_tensor("qrhs", [128, KC, N], qdt, kind="ExternalInput")
    lwrow_d = nc.dram_tensor("lwrow", [1, N], F16, kind="ExternalInput")
    ybc_d = nc.dram_tensor("ybc", [128, N], F16, kind="ExternalInput")
    colid_d = nc.dram_tensor("colid", [128, 128], F16, kind="ExternalInput")
    pidx_d = nc.dram_tensor("pidx", [128, 1], F32, kind="ExternalInput")
    yrow_d = nc.dram_tensor("yrow", [128, NB], F32, kind="ExternalInput")
    crow_d = nc.dram_tensor("crow", [128, NB], F32, kind="ExternalInput")
    dinv_d = nc.dram_tensor("dinv", [128, NB], F32, kind="ExternalInput")
    kr_d = nc.dram_tensor("kr", [NB, 128, KP * D], F16, kind="ExternalInput")
    qr_d = nc.dram_tensor("qr", [NB, 128, D], F16, kind="ExternalInput")
    out_d = nc.dram_tensor("out", [1, 1], F32, kind="ExternalOutput")

    eng = {"vector": nc.vector, "gpsimd": nc.gpsimd}
    ke = eng[k_eng]

    with tile.TileContext(nc) as tc, ExitStack() as ctx:
        const = ctx.enter_context(tc.tile_pool(name="const", bufs=1))
        rh_pool = ctx.enter_context(tc.tile_pool(name="rh", bufs=2))
        psum_pool = ctx.enter_context(tc.tile_pool(name="ps", bufs=2, space="PSUM"))
        ew_pool = ctx.enter_context(tc.tile_pool(name="ew", bufs=3))
        dump_pool = ctx.enter_context(tc.tile_pool(name="dmp", bufs=3))
        k_pool = ctx.enter_context(tc.tile_pool(name="kp", bufs=2))
        q_pool = ctx.enter_context(tc.tile_pool(name="qp", bufs=2))
        kscr_pool = ctx.enter_context(tc.tile_pool(name="ks", bufs=2))

        # ---- resident inputs (priority order for DMA) --------------------
        qlhs = const.tile([128, KC, NL], qdt, tag="qlhs")
        nc.sync.dma_start(qlhs[:, :, :], qlhs_d[:, :, :])
        rhs0 = const.tile([128, KC, TW], qdt, tag="rhs0")
        nc.sync.dma_start(rhs0[:, :, :], qrhs_d[:, :, 0:TW])
        lwrow = const.tile([1, N], F16, tag="lwrow")
        nc.sync.dma_start(lwrow[:, :], lwrow_d[:, :])
        colid = const.tile([128, 128], F16, tag="colid")
        nc.sync.dma_start(colid[:, :], colid_d[:, :])
        pidx = const.tile([128, 1], F32, tag="pidx")
        nc.sync.dma_start(pidx[:, :], pidx_d[:, :])
        yrow = const.tile([128, NB], F32, tag="yrow")
        nc.sync.dma_start(yrow[:, :], yrow_d[:, :])
        crow = const.tile([128, NB], F32, tag="crow")
        nc.sync.dma_start(crow[:, :], crow_d[:, :])
        dinv = const.tile([128, NB], F32, tag="dinv")
        nc.sync.dma_start(dinv[:, :], dinv_d[:, :])
        ybc = const.tile([128, N], F16, tag="ybc")
        nc.sync.dma_start(ybc[:, :], ybc_d[:, :])

        ones_k1 = const.tile([1, 128], F16, tag="ones_k1")
        nc.vector.memset(ones_k1[:, :], 1.0)
        ones_col = const.tile([128, 1], F32, tag="ones_col")
        nc.vector.memset(ones_col[:, :], 1.0)

        # accumulator slots
        aslt = const.tile([128, NB * NS], F32, tag="aslt")
        bslt = const.tile([128, NB * NS], F32, tag="bslt")
        kss = const.tile([128, NB * KP], F32, tag="kss")
        kpos = const.tile([128, NB], F32, tag="kpos")

        kpb = (NB + NS - 1) // NS  # k-path blocks per column tile

        # ---- main loop: score slab ---------------------------------------
        for s in range(NS):
            if s == 0:
                rhs = rhs0
            else:
                rhs = rh_pool.tile([128, KC, TW], qdt, tag="rh", name=f"rhs{s}")
                nc.sync.dma_start(rhs[:, :, :], qrhs_d[:, :, s * TW:(s + 1) * TW])

            # interleave k-path blocks with the main loop
            for j in range(kpb):
                bk = s * kpb + j
                if bk >= NB:
                    continue
                kt = k_pool.tile([128, KP * D], F16, tag="kt")
                nc.sync.dma_start(kt[:, :], kr_d[bk, :, :])
                qt = q_pool.tile([128, D], F16, tag="qt")
                nc.sync.dma_start(qt[:, :], qr_d[bk, :, :])
                for kk in range(KP):
                    kscr = kscr_pool.tile([128, D], F16, tag="kscr")
                    ke.scalar_tensor_tensor(
                        kscr[:, :], kt[:, kk * D:(kk + 1) * D], 1.0,
                        qt[:, :],
                        op0=mybir.AluOpType.mult, op1=mybir.AluOpType.mult,
                        accum_out=kss[:, bk * KP + kk: bk * KP + kk + 1])
                ksse = const.tile([128, KP], F32, tag=f"ksse{bk}")
                nc.scalar.activation(
                    ksse[:, :],
                    kss[:, bk * KP:(bk + 1) * KP],
                    mybir.ActivationFunctionType.Exp, scale=float(1.0 / TAU),
                    accum_out=kpos[:, bk:bk + 1])

            for b in range(NB):
                ps = psum_pool.tile([128, TW], F32, tag="ps")
                for nch in range(NCH):
                    o = ps[:, nch * 512:(nch + 1) * 512]
                    if mode == "fp8dr":
                        for c2 in range(KC // 2):
                            nc.tensor.matmul(
                                o,
                                qlhs[:, 2 * c2:2 * c2 + 2, b * 128:(b + 1) * 128],
                                rhs[:, 2 * c2:2 * c2 + 2, nch * 512:(nch + 1) * 512],
                                start=(c2 == 0), stop=False,
                                perf_mode=mybir.MatmulPerfMode.DoubleRow)
                    else:
                        for c in range(KC):
                            nc.tensor.matmul(
                                o,
                                qlhs[:, c, b * 128:(b + 1) * 128],
                                rhs[:, c, nch * 512:(nch + 1) * 512],
                                start=(c == 0), stop=False)
                    # rank-1: add lw_j (log class weight) to every score
                    nc.tensor.matmul(
                        o,
                        ones_k1[0:1, :],
                        lwrow[0:1, s * TW + nch * 512: s * TW + (nch + 1) * 512],
                        start=False, stop=True)
                if s == 0:
                    # zero the 128 diagonal scores of this row block in PSUM
                    nc.vector.scalar_tensor_tensor(
                        ps[:, b * 128:(b + 1) * 128],
                        colid[:, :], pidx[:, 0:1],
                        ps[:, b * 128:(b + 1) * 128],
                        op0=mybir.AluOpType.not_equal, op1=mybir.AluOpType.mult)
                ew = ew_pool.tile([128, TW], F32)
                nc.scalar.activation(ew[:, :], ps[:, :],
                                     mybir.ActivationFunctionType.Exp,
                                     scale=exp_scale,
                                     accum_out=aslt[:, (b * NS + s):(b * NS + s) + 1])
                # B: same-class weighted row-sum
                d2 = dump_pool.tile([128, TW], F16)
                nc.vector.scalar_tensor_tensor(
                    d2[:, :], ybc[:, s * TW:(s + 1) * TW], yrow[:, b:b + 1],
                    ew[:, :],
                    op0=mybir.AluOpType.is_equal, op1=mybir.AluOpType.mult,
                    accum_out=bslt[:, (b * NS + s):(b * NS + s) + 1])

        # ---- finalize (vectorized over [128, NB]) ------------------------
        acolM = const.tile([128, NB], F32, tag="acolM")
        bcolM = const.tile([128, NB], F32, tag="bcolM")
        for b in range(NB):
            nc.vector.tensor_reduce(acolM[:, b:b + 1], aslt[:, b * NS:(b + 1) * NS],
                                    mybir.AxisListType.X, mybir.AluOpType.add)
            nc.vector.tensor_reduce(bcolM[:, b:b + 1], bslt[:, b * NS:(b + 1) * NS],
                                    mybir.AxisListType.X, mybir.AluOpType.add)
        bm1 = const.tile([128, NB], F32, tag="bm1")
        nc.vector.tensor_scalar_add(bm1[:, :], bcolM[:, :], -1.0)
        numM = const.tile([128, NB], F32, tag="numM")
        # numM = kpos + crow * (B - 1)
        nc.vector.scalar_tensor_tensor(
            numM[:, :], bm1[:, :], 1.0, crow[:, :],
            op0=mybir.AluOpType.mult, op1=mybir.AluOpType.mult)
        nc.vector.tensor_add(numM[:, :], numM[:, :], kpos[:, :])
        denM = const.tile([128, NB], F32, tag="denM")
        nc.vector.tensor_sub(denM[:, :], acolM[:, :], bcolM[:, :])
        denL = const.tile([128, NB], F32, tag="denL")
        nc.scalar.activation(denL[:, :], denM[:, :], mybir.ActivationFunctionType.Ln)
        numL = const.tile([128, NB], F32, tag="numL")
        nc.scalar.activation(numL[:, :], numM[:, :], mybir.ActivationFunctionType.Ln)
        diffM = const.tile([128, NB], F32, tag="diffM")
        nc.vector.tensor_sub(diffM[:, :], denL[:, :], numL[:, :])
        losscol = const.tile([128, NB], F32, tag="losscol")
        nc.vector.tensor_mul(losscol[:, :], diffM[:, :], dinv[:, :])

        # ---- reduce to a single partial ----------------------------------
        lsum = const.tile([128, 1], F32, tag="lsum")
        nc.vector.tensor_reduce(lsum[:, :], losscol[:, :],
                                mybir.AxisListType.X, mybir.AluOpType.add)
        psf = psum_pool.tile([128, TW], F32, tag="ps")
        nc.tensor.matmul(psf[0:1, 0:1], lsum[:, :],
                         ones_col[:, :], start=True, stop=True)
        outsb = const.tile([1, 1], F32, tag="outsb")
        nc.scalar.copy(outsb[0:1, 0:1], psf[0:1, 0:1])
        nc.sync.dma_start(out_d[:, :], outsb[0:1, 0:1])

    nc.compile()
    return nc


# ---------------------------------------------------------------------------
# host-side marshalling
# ---------------------------------------------------------------------------

def make_inputs(q, k, y, cfg: Cfg):
    """Build the per-core input maps (layout/replication marshalling)."""
    N, D, KP, TW = cfg.N, cfg.D, cfg.KP, cfg.TW
    NL, NB, NS, KC = cfg.NL, cfg.NB, cfg.NS, cfg.KC
    q = np.asarray(q, dtype=np.float32)
    k = np.asarray(k, dtype=np.float32)
    y = np.asarray(y).astype(np.int64)

    qdt_np = {"fp8dr": ml_dtypes.float8_e4m3fn,
              "bf16": ml_dtypes.bfloat16,
              "f32r": np.float32}[cfg.mode]
    qs = (q * cfg.prescale).astype(qdt_np)          # [N, D] quantized

    counts = np.bincount(y, minlength=NUM_CLASSES)
    w16 = (1.0 / counts[y].astype(np.float64)).astype(np.float16)  # [N]
    # lw = ln(w) * prescale^2 * TAU, so exp(scale * psum) folds in w exactly
    lw16 = (np.log(w16.astype(np.float64))
            * cfg.prescale * cfg.prescale * TAU).astype(np.float16)
    ylab = (y + 1).astype(np.float16)                              # labels 1..C

    colid = np.broadcast_to(np.arange(128, dtype=np.float16)[None, :],
                            (128, 128)).copy()
    pidx = np.arange(128, dtype=np.float32).reshape(128, 1).copy()

    in_maps = []
    for r in range(NCORES):
        rows = slice(r * NL, (r + 1) * NL)
        rot = (np.arange(N) + r * NL) % N

        # lhsT chunks: qlhs[p, c, i] = qs[r*NL+i, c*128+p]
        qlhs = np.ascontiguousarray(
            qs[rows].T.reshape(KC, 128, NL).transpose(1, 0, 2))
        # rhs chunks, rotated: qrhs[p, c, j] = qs[rot(j), c*128+p]
        qrhs = np.ascontiguousarray(
            qs[rot].T.reshape(KC, 128, N).transpose(1, 0, 2))

        ybc = np.broadcast_to(ylab[rot][None, :], (128, N)).copy()
        lwrow = lw16[rot].reshape(1, N).copy()

        yloc = y[rows]
        yrowm = np.ascontiguousarray(
            (yloc + 1).astype(np.float32).reshape(NB, 128).T)
        crowm = np.ascontiguousarray(
            counts[yloc].astype(np.float32).reshape(NB, 128).T)
        dinvm = np.ascontiguousarray(
            (1.0 / (counts[yloc] - 1 + KP)).astype(np.float32).reshape(NB, 128).T)

        kr = np.ascontiguousarray(
            k[rows].reshape(NB, 128, KP * D)).astype(np.float16)
        qr = np.ascontiguousarray(
            q[rows].reshape(NB, 128, D)).astype(np.float16)

        in_maps.append({
            "qlhs": qlhs, "qrhs": qrhs, "lwrow": lwrow,
            "ybc": ybc, "colid": colid, "pidx": pidx,
            "yrow": yrowm, "crow": crowm, "dinv": dinvm,
            "kr": kr, "qr": qr,
        })
    return in_maps


_CACHE = {}


def _get_nc(mode="bf16"):
    if mode not in _CACHE:
        cfg = Cfg(mode=mode)
        _CACHE[mode] = (cfg, build_bass(cfg))
    return _CACHE[mode]


def kernel(q, k, y, trace=False, mode="bf16"):
    cfg, nc = _get_nc(mode)
    in_maps = make_inputs(q, k, y, cfg)
    res = run_bass_kernel_spmd(nc, in_maps, core_ids=list(range(NCORES)),
                               trace=trace)
    total = np.sum([res.results[r]["out"][0, 0] for r in range(NCORES)],
                   dtype=np.float64)
    out = np.asarray(total / cfg.N, dtype=np.float32)
    if trace:
        kernel.last_results = res
    return out


# revision 14
# speedup vs baseline: 2.3515x; 1.0250x over previous
"""Trainium2 Bass kernel for the supervised-contrastive loss (nn_KCL_69784628626020).

Strategy (8 NeuronCores, SPMD):
  - Shard anchors (rows of q, k, y) across cores: 1024 rows/core.
  - Each core computes its [1024, 8192] slab of the score matrix
    S = q_loc @ q_full^T on the tensor engine.  In fp8 mode the q operands
    are prescaled by 16 and cast to e4m3, and pairs of 128-deep contraction
    chunks run in DoubleRow perf mode (2 cols/cycle).
  - Per-column class weights w_j = 1/count(y_j) come from a host-side
    bincount (input marshalling); their logs are folded into the scores as
    a rank-1 (ones x lw) matmul into PSUM, so the scalar engine's
    exp(scale*PSUM) directly produces EW_ij = exp(s_ij/TAU) * w_j and its
    free accum_out gives the weighted row sum A_i = sum_j EW_ij per tile.
  - The column space of each core is ROTATED by r*NL so the diagonal block
    always lands in column-tile s=0.  There, the 128 diagonal scores per
    row block are zeroed IN PSUM by one small [128,128] DVE op (compare
    column-iota vs partition-iota, multiply), making the diagonal's
    post-exp contribution exactly 1.0 in every reduction.
  - Per (s,b) tile after the exp, ONE DVE scalar_tensor_tensor computes
        B_i += sum_{y_j==y_i} EW_ij     (same-class weighted sum)
  - Finalize per row (c = class count, w16 = fp16(1/c)):
        den_i = A_i - B_i               (diagonal 1s cancel exactly)
        num_i = kpos_i + c_i * (B_i - 1)
        loss_i = (ln den_i - ln num_i) / (c_i - 1 + K)
  - kpos_i = sum_k exp(q_i . k_ik / TAU) via DVE multiply-reduce per k
    (fp16 inputs), exp+accum on the scalar engine.
  - Final mean: per-core partial sum via a ones-matmul partition reduction;
    host adds the 8 partials (the unshard step).
"""

import numpy as np
from contextlib import ExitStack

import concourse.bass as bass
import concourse.bacc as bacc
import concourse.tile as tile
from concourse import mybir
from concourse.bass_utils import run_bass_kernel_spmd
import ml_dtypes

F32 = mybir.dt.float32
F32R = mybir.dt.float32r
F16 = mybir.dt.float16
BF16 = mybir.dt.bfloat16
FP8 = mybir.dt.float8e4

TAU = 0.07
NCORES = 8
NUM_CLASSES = 1000


class Cfg:
    def __init__(self, N=8192, D=512, KP=8, TW=2048, mode="bf16"):
        self.N = N            # total rows (anchors)
        self.D = D            # feature dim
        self.KP = KP          # external positives per anchor
        self.TW = TW          # column tile width
        self.mode = mode      # "fp8dr" | "bf16" | "f32r"
        self.NL = N // NCORES     # rows per core
        self.NB = self.NL // 128  # row blocks per core
        self.NS = N // TW         # column tiles
        self.KC = D // 128        # contraction chunks
        assert self.NL % 128 == 0 and N % TW == 0 and D % 128 == 0
        assert TW % 512 == 0 and self.NL <= TW
        self.TWH = TW // 2        # PSUM half-tile width (4-deep PSUM pipeline)
        assert self.TWH % 512 == 0
        self.NCH = self.TWH // 512  # 512-wide matmul chunks per PSUM half
        # prescale keeps fp8 q elements in the e4m3 normal range
        self.prescale = 16.0 if mode == "fp8dr" else 1.0


def build_bass(cfg: Cfg, k_eng="vector"):
    N, D, KP, TW = cfg.N, cfg.D, cfg.KP, cfg.TW
    NL, NB, NS, KC, NCH = cfg.NL, cfg.NB, cfg.NS, cfg.KC, cfg.NCH
    TWH = cfg.TWH
    NS2 = NS * 2              # accumulation slots per row block (half tiles)
    mode = cfg.mode
    qdt = {"fp8dr": FP8, "bf16": BF16, "f32r": F32R}[mode]
    exp_scale = float(1.0 / (cfg.prescale * cfg.prescale * TAU))

    nc = bacc.Bacc("TRN2", target_bir_lowering=False, debug=False,
                   num_devices=NCORES)

    # ---- kernel I/O -------------------------------------------------------
    qlhs_d = nc.dram_tensor("qlhs", [128, KC, NL], qdt, kind="ExternalInput")
    qrhs_d = nc.dram_tensor("qrhs", [128, KC, N], qdt, kind="ExternalInput")
    lwrow_d = nc.dram_tensor("lwrow", [1, N], F16, kind="ExternalInput")
    ybc_d = nc.dram_tensor("ybc", [128, N], F16, kind="ExternalInput")
    colid_d = nc.dram_tensor("colid", [128, 128], F16, kind="ExternalInput")
    pidx_d = nc.dram_tensor("pidx", [128, 1], F32, kind="ExternalInput")
    yrow_d = nc.dram_tensor("yrow", [128, NB], F32, kind="ExternalInput")
    crow_d = nc.dram_tensor("crow", [128, NB], F32, kind="ExternalInput")
    dinv_d = nc.dram_tensor("dinv", [128, NB], F32, kind="ExternalInput")
    kr_d = nc.dram_tensor("kr", [NB, 128, KP * D], F16, kind="ExternalInput")
    qr_d = nc.dram_tensor("qr", [NB, 128, D], F16, kind="ExternalInput")
    out_d = nc.dram_tensor("out", [1, 1], F32, kind="ExternalOutput")

    eng = {"vector": nc.vector, "gpsimd": nc.gpsimd}
    ke = eng[k_eng]

    with tile.TileContext(nc) as tc, ExitStack() as ctx:
        const = ctx.enter_context(tc.tile_pool(name="const", bufs=1))
        rh_pool = ctx.enter_context(tc.tile_pool(name="rh", bufs=2))
        psum_pool = ctx.enter_context(tc.tile_pool(name="ps", bufs=4, space="PSUM"))
        ew_pool = ctx.enter_context(tc.tile_pool(name="ew", bufs=4))
        dump_pool = ctx.enter_context(tc.tile_pool(name="dmp", bufs=3))
        k_pool = ctx.enter_context(tc.tile_pool(name="kp", bufs=2))
        q_pool = ctx.enter_context(tc.tile_pool(name="qp", bufs=2))
        kscr_pool = ctx.enter_context(tc.tile_pool(name="ks", bufs=2))

        # ---- resident inputs (priority order for DMA) --------------------
        qlhs = const.tile([128, KC, NL], qdt, tag="qlhs")
        nc.sync.dma_start(qlhs[:, :, :], qlhs_d[:, :, :])
        rhs0 = const.tile([128, KC, TW], qdt, tag="rhs0")
        nc.sync.dma_start(rhs0[:, :, :], qrhs_d[:, :, 0:TW])
        lwrow = const.tile([1, N], F16, tag="lwrow")
        nc.sync.dma_start(lwrow[:, :], lwrow_d[:, :])
        colid = const.tile([128, 128], F16, tag="colid")
        nc.sync.dma_start(colid[:, :], colid_d[:, :])
        pidx = const.tile([128, 1], F32, tag="pidx")
        nc.sync.dma_start(pidx[:, :], pidx_d[:, :])
        yrow = const.tile([128, NB], F32, tag="yrow")
        nc.sync.dma_start(yrow[:, :], yrow_d[:, :])
        crow = const.tile([128, NB], F32, tag="crow")
        nc.sync.dma_start(crow[:, :], crow_d[:, :])
        dinv = const.tile([128, NB], F32, tag="dinv")
        nc.sync.dma_start(dinv[:, :], dinv_d[:, :])
        ybc = const.tile([128, N], F16, tag="ybc")
        nc.sync.dma_start(ybc[:, :], ybc_d[:, :])

        ones_k1 = const.tile([1, 128], F16, tag="ones_k1")
        nc.vector.memset(ones_k1[:, :], 1.0)
        ones_col = const.tile([128, 1], F32, tag="ones_col")
        nc.vector.memset(ones_col[:, :], 1.0)

        # accumulator slots
        aslt = const.tile([128, NB * NS2], F32, tag="aslt")
        bslt = const.tile([128, NB * NS2], F32, tag="bslt")
        kss = const.tile([128, NB * KP], F32, tag="kss")
        kpos = const.tile([128, NB], F32, tag="kpos")

        kpb = (NB + NS - 1) // NS  # k-path blocks per column tile

        # ---- main loop: score slab ---------------------------------------
        for s in range(NS):
            if s == 0:
                rhs = rhs0
            else:
                rhs = rh_pool.tile([128, KC, TW], qdt, tag="rh", name=f"rhs{s}")
                nc.sync.dma_start(rhs[:, :, :], qrhs_d[:, :, s * TW:(s + 1) * TW])

            # interleave k-path blocks with the main loop
            for j in range(kpb):
                bk = s * kpb + j
                if bk >= NB:
                    continue
                kt = k_pool.tile([128, KP * D], F16, tag="kt")
                nc.sync.dma_start(kt[:, :], kr_d[bk, :, :])
                qt = q_pool.tile([128, D], F16, tag="qt")
                nc.sync.dma_start(qt[:, :], qr_d[bk, :, :])
                for kk in range(KP):
                    kscr = kscr_pool.tile([128, D], F16, tag="kscr")
                    ke.scalar_tensor_tensor(
                        kscr[:, :], kt[:, kk * D:(kk + 1) * D], 1.0,
                        qt[:, :],
                        op0=mybir.AluOpType.mult, op1=mybir.AluOpType.mult,
                        accum_out=kss[:, bk * KP + kk: bk * KP + kk + 1])
                ksse = const.tile([128, KP], F32, tag=f"ksse{bk}")
                nc.scalar.activation(
                    ksse[:, :],
                    kss[:, bk * KP:(bk + 1) * KP],
                    mybir.ActivationFunctionType.Exp, scale=float(1.0 / TAU),
                    accum_out=kpos[:, bk:bk + 1])

            for b in range(NB):
                for h in range(2):
                    hc0 = h * TWH                 # column offset within tile
                    ps = psum_pool.tile([128, TWH], F32, tag="ps")
                    for nch in range(NCH):
                        o = ps[:, nch * 512:(nch + 1) * 512]
                        r0 = hc0 + nch * 512
                        if mode == "fp8dr":
                            for c2 in range(KC // 2):
                                nc.tensor.matmul(
                                    o,
                                    qlhs[:, 2 * c2:2 * c2 + 2, b * 128:(b + 1) * 128],
                                    rhs[:, 2 * c2:2 * c2 + 2, r0:r0 + 512],
                                    start=(c2 == 0), stop=False,
                                    perf_mode=mybir.MatmulPerfMode.DoubleRow)
                        else:
                            for c in range(KC):
                                nc.tensor.matmul(
                                    o,
                                    qlhs[:, c, b * 128:(b + 1) * 128],
                                    rhs[:, c, r0:r0 + 512],
                                    start=(c == 0), stop=False)
                        # rank-1: add lw_j (log class weight) to every score
                        nc.tensor.matmul(
                            o,
                            ones_k1[0:1, :],
                            lwrow[0:1, s * TW + r0: s * TW + r0 + 512],
                            start=False, stop=True)
                    hd = (b * 128) // TWH
                    if s == 0 and h == hd:
                        # zero the 128 diagonal scores of this row block
                        dc = b * 128 - hd * TWH
                        nc.vector.scalar_tensor_tensor(
                            ps[:, dc:dc + 128],
                            colid[:, :], pidx[:, 0:1],
                            ps[:, dc:dc + 128],
                            op0=mybir.AluOpType.not_equal,
                            op1=mybir.AluOpType.mult)
                    slot = b * NS2 + 2 * s + h
                    ew = ew_pool.tile([128, TWH], F32)
                    nc.scalar.activation(ew[:, :], ps[:, :],
                                         mybir.ActivationFunctionType.Exp,
                                         scale=exp_scale,
                                         accum_out=aslt[:, slot:slot + 1])
                    # B: same-class weighted row-sum
                    d2 = dump_pool.tile([128, TWH], F16)
                    nc.vector.scalar_tensor_tensor(
                        d2[:, :],
                        ybc[:, s * TW + hc0: s * TW + hc0 + TWH],
                        yrow[:, b:b + 1],
                        ew[:, :],
                        op0=mybir.AluOpType.is_equal, op1=mybir.AluOpType.mult,
                        accum_out=bslt[:, slot:slot + 1])

        # ---- finalize (vectorized over [128, NB]) ------------------------
        acolM = const.tile([128, NB], F32, tag="acolM")
        bcolM = const.tile([128, NB], F32, tag="bcolM")
        for b in range(NB):
            nc.vector.tensor_reduce(acolM[:, b:b + 1], aslt[:, b * NS2:(b + 1) * NS2],
                                    mybir.AxisListType.X, mybir.AluOpType.add)
            nc.vector.tensor_reduce(bcolM[:, b:b + 1], bslt[:, b * NS2:(b + 1) * NS2],
                                    mybir.AxisListType.X, mybir.AluOpType.add)
        bm1 = const.tile([128, NB], F32, tag="bm1")
        nc.vector.tensor_scalar_add(bm1[:, :], bcolM[:, :], -1.0)
        numM = const.tile([128, NB], F32, tag="numM")
        # numM = kpos + crow * (B - 1)
        nc.vector.scalar_tensor_tensor(
            numM[:, :], bm1[:, :], 1.0, crow[:, :],
            op0=mybir.AluOpType.mult, op1=mybir.AluOpType.mult)
        nc.vector.tensor_add(numM[:, :], numM[:, :], kpos[:, :])
        denM = const.tile([128, NB], F32, tag="denM")
        nc.vector.tensor_sub(denM[:, :], acolM[:, :], bcolM[:, :])
        denL = const.tile([128, NB], F32, tag="denL")
        nc.scalar.activation(denL[:, :], denM[:, :], mybir.ActivationFunctionType.Ln)
        numL = const.tile([128, NB], F32, tag="numL")
        nc.scalar.activation(numL[:, :], numM[:, :], mybir.ActivationFunctionType.Ln)
        diffM = const.tile([128, NB], F32, tag="diffM")
        nc.vector.tensor_sub(diffM[:, :], denL[:, :], numL[:, :])
        losscol = const.tile([128, NB], F32, tag="losscol")
        nc.vector.tensor_mul(losscol[:, :], diffM[:, :], dinv[:, :])

        # ---- reduce to a single partial ----------------------------------
        lsum = const.tile([128, 1], F32, tag="lsum")
        nc.vector.tensor_reduce(lsum[:, :], losscol[:, :],
                                mybir.AxisListType.X, mybir.AluOpType.add)
        psf = psum_pool.tile([128, TWH], F32, tag="ps")
        nc.tensor.matmul(psf[0:1, 0:1], lsum[:, :],
                         ones_col[:, :], start=True, stop=True)
        outsb = const.tile([1, 1], F32, tag="outsb")
        nc.scalar.copy(outsb[0:1, 0:1], psf[0:1, 0:1])
        nc.sync.dma_start(out_d[:, :], outsb[0:1, 0:1])

    nc.compile()
    return nc


# ---------------------------------------------------------------------------
# host-side marshalling
# ---------------------------------------------------------------------------

def make_inputs(q, k, y, cfg: Cfg):
    """Build the per-core input maps (layout/replication marshalling)."""
    N, D, KP, TW = cfg.N, cfg.D, cfg.KP, cfg.TW
    NL, NB, NS, KC = cfg.NL, cfg.NB, cfg.NS, cfg.KC
    q = np.asarray(q, dtype=np.float32)
    k = np.asarray(k, dtype=np.float32)
    y = np.asarray(y).astype(np.int64)

    qdt_np = {"fp8dr": ml_dtypes.float8_e4m3fn,
              "bf16": ml_dtypes.bfloat16,
              "f32r": np.float32}[cfg.mode]
    qs = (q * cfg.prescale).astype(qdt_np)          # [N, D] quantized

    counts = np.bincount(y, minlength=NUM_CLASSES)
    w16 = (1.0 / counts[y].astype(np.float64)).astype(np.float16)  # [N]
    # lw = ln(w) * prescale^2 * TAU, so exp(scale * psum) folds in w exactly
    lw16 = (np.log(w16.astype(np.float64))
            * cfg.prescale * cfg.prescale * TAU).astype(np.float16)
    ylab = (y + 1).astype(np.float16)                              # labels 1..C

    colid = np.broadcast_to(np.arange(128, dtype=np.float16)[None, :],
                            (128, 128)).copy()
    pidx = np.arange(128, dtype=np.float32).reshape(128, 1).copy()

    in_maps = []
    for r in range(NCORES):
        rows = slice(r * NL, (r + 1) * NL)
        rot = (np.arange(N) + r * NL) % N

        # lhsT chunks: qlhs[p, c, i] = qs[r*NL+i, c*128+p]
        qlhs = np.ascontiguousarray(
            qs[rows].T.reshape(KC, 128, NL).transpose(1, 0, 2))
        # rhs chunks, rotated: qrhs[p, c, j] = qs[rot(j), c*128+p]
        qrhs = np.ascontiguousarray(
            qs[rot].T.reshape(KC, 128, N).transpose(1, 0, 2))

        ybc = np.broadcast_to(ylab[rot][None, :], (128, N)).copy()
        lwrow = lw16[rot].reshape(1, N).copy()

        yloc = y[rows]
        yrowm = np.ascontiguousarray(
            (yloc + 1).astype(np.float32).reshape(NB, 128).T)
        crowm = np.ascontiguousarray(
            counts[yloc].astype(np.float32).reshape(NB, 128).T)
        dinvm = np.ascontiguousarray(
            (1.0 / (counts[yloc] - 1 + KP)).astype(np.float32).reshape(NB, 128).T)

        kr = np.ascontiguousarray(
            k[rows].reshape(NB, 128, KP * D)).astype(np.float16)
        qr = np.ascontiguousarray(
            q[rows].reshape(NB, 128, D)).astype(np.float16)

        in_maps.append({
            "qlhs": qlhs, "qrhs": qrhs, "lwrow": lwrow,
            "ybc": ybc, "colid": colid, "pidx": pidx,
            "yrow": yrowm, "crow": crowm, "dinv": dinvm,
            "kr": kr, "qr": qr,
        })
    return in_maps


_CACHE = {}


def _get_nc(mode="bf16"):
    if mode not in _CACHE:
        cfg = Cfg(mode=mode)
        _CACHE[mode] = (cfg, build_bass(cfg))
    return _CACHE[mode]


def kernel(q, k, y, trace=False, mode="bf16"):
    cfg, nc = _get_nc(mode)
    in_maps = make_inputs(q, k, y, cfg)
    res = run_bass_kernel_spmd(nc, in_maps, core_ids=list(range(NCORES)),
                               trace=trace)
    total = np.sum([res.results[r]["out"][0, 0] for r in range(NCORES)],
                   dtype=np.float64)
    out = np.asarray(total / cfg.N, dtype=np.float32)
    if trace:
        kernel.last_results = res
    return out


# revision 21
# speedup vs baseline: 2.6043x; 1.1075x over previous
"""Trainium2 Bass kernel for the supervised-contrastive loss (nn_KCL_69784628626020).

Strategy (8 NeuronCores, SPMD):
  - Shard anchors (rows of q, k, y) across cores: 1024 rows/core.
  - Each core computes its [1024, 8192] slab of the score matrix
    S = q_loc @ q_full^T on the tensor engine.  In fp8 mode the q operands
    are prescaled by 16 and cast to e4m3, and pairs of 128-deep contraction
    chunks run in DoubleRow perf mode (2 cols/cycle).
  - Per-column class weights w_j = 1/count(y_j) come from a host-side
    bincount (input marshalling); their logs are folded into the scores as
    a rank-1 (ones x lw) matmul into PSUM, so the scalar engine's
    exp(scale*PSUM) directly produces EW_ij = exp(s_ij/TAU) * w_j and its
    free accum_out gives the weighted row sum A_i = sum_j EW_ij per tile.
  - The column space of each core is ROTATED by r*NL so the diagonal block
    always lands in column-tile s=0.  There, the 128 diagonal scores per
    row block are zeroed IN PSUM by one small [128,128] DVE op (compare
    column-iota vs partition-iota, multiply), making the diagonal's
    post-exp contribution exactly 1.0 in every reduction.
  - Per (s,b) tile after the exp, ONE DVE scalar_tensor_tensor computes
        B_i += sum_{y_j==y_i} EW_ij     (same-class weighted sum)
  - Finalize per row (c = class count, w16 = fp16(1/c)):
        den_i = A_i - B_i               (diagonal 1s cancel exactly)
        num_i = kpos_i + c_i * (B_i - 1)
        loss_i = (ln den_i - ln num_i) / (c_i - 1 + K)
  - kpos_i = sum_k exp(q_i . k_ik / TAU) via DVE multiply-reduce per k
    (fp16 inputs), exp+accum on the scalar engine.
  - Final mean: per-core partial sum via a ones-matmul partition reduction;
    host adds the 8 partials (the unshard step).
"""

import numpy as np
from contextlib import ExitStack

import concourse.bass as bass
import concourse.bacc as bacc
import concourse.tile as tile
from concourse import mybir
from concourse.bass_utils import run_bass_kernel_spmd
import ml_dtypes

F32 = mybir.dt.float32
F32R = mybir.dt.float32r
F16 = mybir.dt.float16
BF16 = mybir.dt.bfloat16
FP8 = mybir.dt.float8e4

TAU = 0.07
NCORES = 8
NUM_CLASSES = 1000


class Cfg:
    def __init__(self, N=8192, D=512, KP=8, TW=2048, mode="bf16"):
        self.N = N            # total rows (anchors)
        self.D = D            # feature dim
        self.KP = KP          # external positives per anchor
        self.TW = TW          # column tile width
        self.mode = mode      # "fp8dr" | "bf16" | "f32r"
        self.NL = N // NCORES     # rows per core
        self.NB = self.NL // 128  # row blocks per core
        self.NS = N // TW         # column tiles
        self.KC = D // 128        # contraction chunks
        assert self.NL % 128 == 0 and N % TW == 0 and D % 128 == 0
        assert TW % 512 == 0 and self.NL <= TW
        self.TWH = TW // 2        # PSUM half-tile width (4-deep PSUM pipeline)
        assert self.TWH % 512 == 0
        self.NCH = self.TWH // 512  # 512-wide matmul chunks per PSUM half
        # prescale keeps fp8 q elements in the e4m3 normal range
        self.prescale = 16.0 if mode == "fp8dr" else 1.0


def build_bass(cfg: Cfg, k_eng="vector"):
    N, D, KP, TW = cfg.N, cfg.D, cfg.KP, cfg.TW
    NL, NB, NS, KC, NCH = cfg.NL, cfg.NB, cfg.NS, cfg.KC, cfg.NCH
    TWH = cfg.TWH
    NS2 = NS * 2              # accumulation slots per row block (half tiles)
    mode = cfg.mode
    qdt = {"fp8dr": FP8, "bf16": BF16, "f32r": F32R}[mode]
    exp_scale = float(1.0 / (cfg.prescale * cfg.prescale * TAU))

    nc = bacc.Bacc("TRN2", target_bir_lowering=False, debug=False,
                   num_devices=NCORES)

    # ---- kernel I/O -------------------------------------------------------
    qlhs_d = nc.dram_tensor("qlhs", [128, KC, NL], qdt, kind="ExternalInput")
    qrhs_d = nc.dram_tensor("qrhs", [128, KC, N], qdt, kind="ExternalInput")
    if mode == "fp8dr":
        # lw split into fp8 hi+lo rows so the rank-1 stays in DoubleRow mode
        # (mixing f16 matmuls into a DR stream costs ~900ns/switch on the PE)
        lwdr_d = nc.dram_tensor("lwdr", [1, 2, N], FP8, kind="ExternalInput")
    else:
        lwrow_d = nc.dram_tensor("lwrow", [1, N], F16, kind="ExternalInput")
    ybc_d = nc.dram_tensor("ybc", [128, N], F16, kind="ExternalInput")
    colid_d = nc.dram_tensor("colid", [128, 128], F16, kind="ExternalInput")
    pidx_d = nc.dram_tensor("pidx", [128, 1], F32, kind="ExternalInput")
    yrow_d = nc.dram_tensor("yrow", [128, NB], F32, kind="ExternalInput")
    crow_d = nc.dram_tensor("crow", [128, NB], F32, kind="ExternalInput")
    dinv_d = nc.dram_tensor("dinv", [128, NB], F32, kind="ExternalInput")
    kr_d = nc.dram_tensor("kr", [NB, 128, KP * D], F16, kind="ExternalInput")
    qr_d = nc.dram_tensor("qr", [NB, 128, D], F16, kind="ExternalInput")
    out_d = nc.dram_tensor("out", [1, 1], F32, kind="ExternalOutput")

    eng = {"vector": nc.vector, "gpsimd": nc.gpsimd}
    ke = eng[k_eng]

    with tile.TileContext(nc) as tc, ExitStack() as ctx:
        const = ctx.enter_context(tc.tile_pool(name="const", bufs=1))
        rh_pool = ctx.enter_context(tc.tile_pool(name="rh", bufs=2))
        psum_pool = ctx.enter_context(tc.tile_pool(name="ps", bufs=4, space="PSUM"))
        ew_pool = ctx.enter_context(tc.tile_pool(name="ew", bufs=4))
        dump_pool = ctx.enter_context(tc.tile_pool(name="dmp", bufs=3))
        k_pool = ctx.enter_context(tc.tile_pool(name="kp", bufs=2))
        q_pool = ctx.enter_context(tc.tile_pool(name="qp", bufs=2))
        kscr_pool = ctx.enter_context(tc.tile_pool(name="ks", bufs=2))

        # ---- resident inputs (priority order for DMA) --------------------
        qlhs = const.tile([128, KC, NL], qdt, tag="qlhs")
        nc.sync.dma_start(qlhs[:, :, :], qlhs_d[:, :, :])
        rhs0 = const.tile([128, KC, TW], qdt, tag="rhs0")
        nc.sync.dma_start(rhs0[:, :, :], qrhs_d[:, :, 0:TW])
        if mode == "fp8dr":
            lwdr = const.tile([1, 2, N], FP8, tag="lwdr")
            nc.sync.dma_start(lwdr[:, :, :], lwdr_d[:, :, :])
        else:
            lwrow = const.tile([1, N], F16, tag="lwrow")
            nc.sync.dma_start(lwrow[:, :], lwrow_d[:, :])
        colid = const.tile([128, 128], F16, tag="colid")
        nc.sync.dma_start(colid[:, :], colid_d[:, :])
        pidx = const.tile([128, 1], F32, tag="pidx")
        nc.sync.dma_start(pidx[:, :], pidx_d[:, :])
        yrow = const.tile([128, NB], F32, tag="yrow")
        nc.sync.dma_start(yrow[:, :], yrow_d[:, :])
        crow = const.tile([128, NB], F32, tag="crow")
        nc.sync.dma_start(crow[:, :], crow_d[:, :])
        dinv = const.tile([128, NB], F32, tag="dinv")
        nc.sync.dma_start(dinv[:, :], dinv_d[:, :])
        ybc = const.tile([128, N], F16, tag="ybc")
        nc.sync.dma_start(ybc[:, :], ybc_d[:, :])

        if mode == "fp8dr":
            ones_k2 = const.tile([1, 2, 128], FP8, tag="ones_k2")
            nc.vector.memset(ones_k2[:, :, :], 1.0)
        else:
            ones_k1 = const.tile([1, 128], F16, tag="ones_k1")
            nc.vector.memset(ones_k1[:, :], 1.0)
        ones_col = const.tile([128, 1], F32, tag="ones_col")
        nc.vector.memset(ones_col[:, :], 1.0)

        # accumulator slots
        aslt = const.tile([128, NB * NS2], F32, tag="aslt")
        bslt = const.tile([128, NB * NS2], F32, tag="bslt")
        kss = const.tile([128, NB * KP], F32, tag="kss")
        kpos = const.tile([128, NB], F32, tag="kpos")

        kpb = (NB + NS - 1) // NS  # k-path blocks per column tile

        # ---- main loop: score slab ---------------------------------------
        for s in range(NS):
            if s == 0:
                rhs = rhs0
            else:
                rhs = rh_pool.tile([128, KC, TW], qdt, tag="rh", name=f"rhs{s}")
                nc.sync.dma_start(rhs[:, :, :], qrhs_d[:, :, s * TW:(s + 1) * TW])

            # interleave k-path blocks with the main loop
            for j in range(kpb):
                bk = s * kpb + j
                if bk >= NB:
                    continue
                kt = k_pool.tile([128, KP * D], F16, tag="kt")
                nc.sync.dma_start(kt[:, :], kr_d[bk, :, :])
                qt = q_pool.tile([128, D], F16, tag="qt")
                nc.sync.dma_start(qt[:, :], qr_d[bk, :, :])
                for kk in range(KP):
                    kscr = kscr_pool.tile([128, D], F16, tag="kscr")
                    ke.scalar_tensor_tensor(
                        kscr[:, :], kt[:, kk * D:(kk + 1) * D], 1.0,
                        qt[:, :],
                        op0=mybir.AluOpType.mult, op1=mybir.AluOpType.mult,
                        accum_out=kss[:, bk * KP + kk: bk * KP + kk + 1])
                ksse = const.tile([128, KP], F32, tag=f"ksse{bk}")
                nc.scalar.activation(
                    ksse[:, :],
                    kss[:, bk * KP:(bk + 1) * KP],
                    mybir.ActivationFunctionType.Exp, scale=float(1.0 / TAU),
                    accum_out=kpos[:, bk:bk + 1])

            for b in range(NB):
                for h in range(2):
                    hc0 = h * TWH                 # column offset within tile
                    ps = psum_pool.tile([128, TWH], F32, tag="ps")
                    for nch in range(NCH):
                        o = ps[:, nch * 512:(nch + 1) * 512]
                        r0 = hc0 + nch * 512
                        if mode == "fp8dr":
                            for c2 in range(KC // 2):
                                nc.tensor.matmul(
                                    o,
                                    qlhs[:, 2 * c2:2 * c2 + 2, b * 128:(b + 1) * 128],
                                    rhs[:, 2 * c2:2 * c2 + 2, r0:r0 + 512],
                                    start=(c2 == 0), stop=False,
                                    perf_mode=mybir.MatmulPerfMode.DoubleRow)
                        else:
                            for c in range(KC):
                                nc.tensor.matmul(
                                    o,
                                    qlhs[:, c, b * 128:(b + 1) * 128],
                                    rhs[:, c, r0:r0 + 512],
                                    start=(c == 0), stop=False)
                        # rank-1: add lw_j (log class weight) to every score
                        if mode == "fp8dr":
                            nc.tensor.matmul(
                                o,
                                ones_k2[0:1, :, :],
                                lwdr[0:1, :, s * TW + r0: s * TW + r0 + 512],
                                start=False, stop=True,
                                perf_mode=mybir.MatmulPerfMode.DoubleRow)
                        else:
                            nc.tensor.matmul(
                                o,
                                ones_k1[0:1, :],
                                lwrow[0:1, s * TW + r0: s * TW + r0 + 512],
                                start=False, stop=True)
                    hd = (b * 128) // TWH
                    if s == 0 and h == hd:
                        # zero the 128 diagonal scores of this row block
                        dc = b * 128 - hd * TWH
                        nc.vector.scalar_tensor_tensor(
                            ps[:, dc:dc + 128],
                            colid[:, :], pidx[:, 0:1],
                            ps[:, dc:dc + 128],
                            op0=mybir.AluOpType.not_equal,
                            op1=mybir.AluOpType.mult)
                    slot = b * NS2 + 2 * s + h
                    ew = ew_pool.tile([128, TWH], F32)
                    nc.scalar.activation(ew[:, :], ps[:, :],
                                         mybir.ActivationFunctionType.Exp,
                                         scale=exp_scale,
                                         accum_out=aslt[:, slot:slot + 1])
                    # B: same-class weighted row-sum
                    d2 = dump_pool.tile([128, TWH], F16)
                    nc.vector.scalar_tensor_tensor(
                        d2[:, :],
                        ybc[:, s * TW + hc0: s * TW + hc0 + TWH],
                        yrow[:, b:b + 1],
                        ew[:, :],
                        op0=mybir.AluOpType.is_equal, op1=mybir.AluOpType.mult,
                        accum_out=bslt[:, slot:slot + 1])

        # ---- finalize (vectorized over [128, NB]) ------------------------
        acolM = const.tile([128, NB], F32, tag="acolM")
        bcolM = const.tile([128, NB], F32, tag="bcolM")
        for b in range(NB):
            nc.vector.tensor_reduce(acolM[:, b:b + 1], aslt[:, b * NS2:(b + 1) * NS2],
                                    mybir.AxisListType.X, mybir.AluOpType.add)
            nc.vector.tensor_reduce(bcolM[:, b:b + 1], bslt[:, b * NS2:(b + 1) * NS2],
                                    mybir.AxisListType.X, mybir.AluOpType.add)
        bm1 = const.tile([128, NB], F32, tag="bm1")
        nc.vector.tensor_scalar_add(bm1[:, :], bcolM[:, :], -1.0)
        numM = const.tile([128, NB], F32, tag="numM")
        # numM = kpos + crow * (B - 1)
        nc.vector.scalar_tensor_tensor(
            numM[:, :], bm1[:, :], 1.0, crow[:, :],
            op0=mybir.AluOpType.mult, op1=mybir.AluOpType.mult)
        nc.vector.tensor_add(numM[:, :], numM[:, :], kpos[:, :])
        denM = const.tile([128, NB], F32, tag="denM")
        nc.vector.tensor_sub(denM[:, :], acolM[:, :], bcolM[:, :])
        denL = const.tile([128, NB], F32, tag="denL")
        nc.scalar.activation(denL[:, :], denM[:, :], mybir.ActivationFunctionType.Ln)
        numL = const.tile([128, NB], F32, tag="numL")
        nc.scalar.activation(numL[:, :], numM[:, :], mybir.ActivationFunctionType.Ln)
        diffM = const.tile([128, NB], F32, tag="diffM")
        nc.vector.tensor_sub(diffM[:, :], denL[:, :], numL[:, :])
        losscol = const.tile([128, NB], F32, tag="losscol")
        nc.vector.tensor_mul(losscol[:, :], diffM[:, :], dinv[:, :])

        # ---- reduce to a single partial ----------------------------------
        lsum = const.tile([128, 1], F32, tag="lsum")
        nc.vector.tensor_reduce(lsum[:, :], losscol[:, :],
                                mybir.AxisListType.X, mybir.AluOpType.add)
        psf = psum_pool.tile([128, TWH], F32, tag="ps")
        nc.tensor.matmul(psf[0:1, 0:1], lsum[:, :],
                         ones_col[:, :], start=True, stop=True)
        outsb = const.tile([1, 1], F32, tag="outsb")
        nc.scalar.copy(outsb[0:1, 0:1], psf[0:1, 0:1])
        nc.sync.dma_start(out_d[:, :], outsb[0:1, 0:1])

    nc.compile()
    return nc


# ---------------------------------------------------------------------------
# host-side marshalling
# ---------------------------------------------------------------------------

def make_inputs(q, k, y, cfg: Cfg):
    """Build the per-core input maps (layout/replication marshalling)."""
    N, D, KP, TW = cfg.N, cfg.D, cfg.KP, cfg.TW
    NL, NB, NS, KC = cfg.NL, cfg.NB, cfg.NS, cfg.KC
    q = np.asarray(q, dtype=np.float32)
    k = np.asarray(k, dtype=np.float32)
    y = np.asarray(y).astype(np.int64)

    qdt_np = {"fp8dr": ml_dtypes.float8_e4m3fn,
              "bf16": ml_dtypes.bfloat16,
              "f32r": np.float32}[cfg.mode]
    qs = (q * cfg.prescale).astype(qdt_np)          # [N, D] quantized

    counts = np.bincount(y, minlength=NUM_CLASSES)
    w16 = (1.0 / counts[y].astype(np.float64)).astype(np.float16)  # [N]
    # lw = ln(w) * prescale^2 * TAU, so exp(scale * psum) folds in w exactly
    lw = (np.log(w16.astype(np.float64))
          * cfg.prescale * cfg.prescale * TAU)
    lw16 = lw.astype(np.float16)
    lw8_hi = lw.astype(ml_dtypes.float8_e4m3fn)
    lw8_lo = (lw - lw8_hi.astype(np.float64)).astype(ml_dtypes.float8_e4m3fn)
    ylab = (y + 1).astype(np.float16)                              # labels 1..C

    colid = np.broadcast_to(np.arange(128, dtype=np.float16)[None, :],
                            (128, 128)).copy()
    pidx = np.arange(128, dtype=np.float32).reshape(128, 1).copy()

    in_maps = []
    for r in range(NCORES):
        rows = slice(r * NL, (r + 1) * NL)
        rot = (np.arange(N) + r * NL) % N

        # lhsT chunks: qlhs[p, c, i] = qs[r*NL+i, c*128+p]
        qlhs = np.ascontiguousarray(
            qs[rows].T.reshape(KC, 128, NL).transpose(1, 0, 2))
        # rhs chunks, rotated: qrhs[p, c, j] = qs[rot(j), c*128+p]
        qrhs = np.ascontiguousarray(
            qs[rot].T.reshape(KC, 128, N).transpose(1, 0, 2))

        ybc = np.broadcast_to(ylab[rot][None, :], (128, N)).copy()

        yloc = y[rows]
        yrowm = np.ascontiguousarray(
            (yloc + 1).astype(np.float32).reshape(NB, 128).T)
        crowm = np.ascontiguousarray(
            counts[yloc].astype(np.float32).reshape(NB, 128).T)
        dinvm = np.ascontiguousarray(
            (1.0 / (counts[yloc] - 1 + KP)).astype(np.float32).reshape(NB, 128).T)

        kr = np.ascontiguousarray(
            k[rows].reshape(NB, 128, KP * D)).astype(np.float16)
        qr = np.ascontiguousarray(
            q[rows].reshape(NB, 128, D)).astype(np.float16)

        imap = {
            "qlhs": qlhs, "qrhs": qrhs,
            "ybc": ybc, "colid": colid, "pidx": pidx,
            "yrow": yrowm, "crow": crowm, "dinv": dinvm,
            "kr": kr, "qr": qr,
        }
        if cfg.mode == "fp8dr":
            imap["lwdr"] = np.stack([lw8_hi[rot], lw8_lo[rot]]).reshape(1, 2, N)
        else:
            imap["lwrow"] = lw16[rot].reshape(1, N).copy()
        in_maps.append(imap)
    return in_maps


_CACHE = {}


def _get_nc(mode="bf16"):
    if mode not in _CACHE:
        cfg = Cfg(mode=mode)
        _CACHE[mode] = (cfg, build_bass(cfg))
    return _CACHE[mode]


def kernel(q, k, y, trace=False, mode="bf16"):
    cfg, nc = _get_nc(mode)
    in_maps = make_inputs(q, k, y, cfg)
    res = run_bass_kernel_spmd(nc, in_maps, core_ids=list(range(NCORES)),
                               trace=trace)
    total = np.sum([res.results[r]["out"][0, 0] for r in range(NCORES)],
                   dtype=np.float64)
    out = np.asarray(total / cfg.N, dtype=np.float32)
    if trace:
        kernel.last_results = res
    return out


# revision 22
# speedup vs baseline: 2.6872x; 1.0318x over previous
"""Trainium2 Bass kernel for the supervised-contrastive loss (nn_KCL_69784628626020).

Strategy (8 NeuronCores, SPMD):
  - Shard anchors (rows of q, k, y) across cores: 1024 rows/core.
  - Each core computes its [1024, 8192] slab of the score matrix
    S = q_loc @ q_full^T on the tensor engine.  In fp8 mode the q operands
    are prescaled by 16 and cast to e4m3, and pairs of 128-deep contraction
    chunks run in DoubleRow perf mode (2 cols/cycle).
  - Per-column class weights w_j = 1/count(y_j) come from a host-side
    bincount (input marshalling); their logs are folded into the scores as
    a rank-1 (ones x lw) matmul into PSUM, so the scalar engine's
    exp(scale*PSUM) directly produces EW_ij = exp(s_ij/TAU) * w_j and its
    free accum_out gives the weighted row sum A_i = sum_j EW_ij per tile.
  - The column space of each core is ROTATED by r*NL so the diagonal block
    always lands in column-tile s=0.  There, the 128 diagonal scores per
    row block are zeroed IN PSUM by one small [128,128] DVE op (compare
    column-iota vs partition-iota, multiply), making the diagonal's
    post-exp contribution exactly 1.0 in every reduction.
  - Per (s,b) tile after the exp, ONE DVE scalar_tensor_tensor computes
        B_i += sum_{y_j==y_i} EW_ij     (same-class weighted sum)
  - Finalize per row (c = class count, w16 = fp16(1/c)):
        den_i = A_i - B_i               (diagonal 1s cancel exactly)
        num_i = kpos_i + c_i * (B_i - 1)
        loss_i = (ln den_i - ln num_i) / (c_i - 1 + K)
  - kpos_i = sum_k exp(q_i . k_ik / TAU) via DVE multiply-reduce per k
    (fp16 inputs), exp+accum on the scalar engine.
  - Final mean: per-core partial sum via a ones-matmul partition reduction;
    host adds the 8 partials (the unshard step).
"""

import numpy as np
from contextlib import ExitStack

import concourse.bass as bass
import concourse.bacc as bacc
import concourse.tile as tile
from concourse import mybir
from concourse.bass_utils import run_bass_kernel_spmd
import ml_dtypes

F32 = mybir.dt.float32
F32R = mybir.dt.float32r
F16 = mybir.dt.float16
BF16 = mybir.dt.bfloat16
FP8 = mybir.dt.float8e4

TAU = 0.07
NCORES = 8
NUM_CLASSES = 1000


class Cfg:
    def __init__(self, N=8192, D=512, KP=8, TW=2048, mode="bf16"):
        self.N = N            # total rows (anchors)
        self.D = D            # feature dim
        self.KP = KP          # external positives per anchor
        self.TW = TW          # column tile width
        self.mode = mode      # "fp8dr" | "bf16" | "f32r"
        self.NL = N // NCORES     # rows per core
        self.NB = self.NL // 128  # row blocks per core
        self.NS = N // TW         # column tiles
        self.KC = D // 128        # contraction chunks
        assert self.NL % 128 == 0 and N % TW == 0 and D % 128 == 0
        assert TW % 512 == 0 and self.NL <= TW
        self.TWH = TW // 2        # PSUM half-tile width (4-deep PSUM pipeline)
        assert self.TWH % 512 == 0
        self.NCH = self.TWH // 512  # 512-wide matmul chunks per PSUM half
        # prescale keeps fp8 q elements in the e4m3 normal range
        self.prescale = 16.0 if mode == "fp8dr" else 1.0


def build_bass(cfg: Cfg, k_eng="vector"):
    N, D, KP, TW = cfg.N, cfg.D, cfg.KP, cfg.TW
    NL, NB, NS, KC, NCH = cfg.NL, cfg.NB, cfg.NS, cfg.KC, cfg.NCH
    TWH = cfg.TWH
    NS2 = NS * 2              # accumulation slots per row block (half tiles)
    mode = cfg.mode
    qdt = {"fp8dr": FP8, "bf16": BF16, "f32r": F32R}[mode]
    exp_scale = float(1.0 / (cfg.prescale * cfg.prescale * TAU))

    nc = bacc.Bacc("TRN2", target_bir_lowering=False, debug=False,
                   num_devices=NCORES)

    # ---- kernel I/O -------------------------------------------------------
    qlhs_d = nc.dram_tensor("qlhs", [128, KC, NL], qdt, kind="ExternalInput")
    qrhs_d = nc.dram_tensor("qrhs", [128, KC, N], qdt, kind="ExternalInput")
    if mode == "fp8dr":
        # lw split into fp8 hi+lo rows so the rank-1 stays in DoubleRow mode
        # (mixing f16 matmuls into a DR stream costs ~900ns/switch on the PE)
        lwdr_d = nc.dram_tensor("lwdr", [1, 2, N], FP8, kind="ExternalInput")
    else:
        lwrow_d = nc.dram_tensor("lwrow", [1, N], F16, kind="ExternalInput")
    ybc_d = nc.dram_tensor("ybc", [128, N], F16, kind="ExternalInput")
    colid_d = nc.dram_tensor("colid", [128, 128], F16, kind="ExternalInput")
    pidx_d = nc.dram_tensor("pidx", [128, 1], F32, kind="ExternalInput")
    yrow_d = nc.dram_tensor("yrow", [128, NB], F32, kind="ExternalInput")
    crow_d = nc.dram_tensor("crow", [128, NB], F32, kind="ExternalInput")
    dinv_d = nc.dram_tensor("dinv", [128, NB], F32, kind="ExternalInput")
    kr_d = nc.dram_tensor("kr", [NB, 128, KP * D], F16, kind="ExternalInput")
    qr_d = nc.dram_tensor("qr", [NB, 128, D], F16, kind="ExternalInput")
    out_d = nc.dram_tensor("out", [1, 1], F32, kind="ExternalOutput")

    eng = {"vector": nc.vector, "gpsimd": nc.gpsimd}
    ke = eng[k_eng]

    with tile.TileContext(nc) as tc, ExitStack() as ctx:
        const = ctx.enter_context(tc.tile_pool(name="const", bufs=1))
        rh_pool = ctx.enter_context(tc.tile_pool(name="rh", bufs=2))
        psum_pool = ctx.enter_context(tc.tile_pool(name="ps", bufs=4, space="PSUM"))
        ew_pool = ctx.enter_context(tc.tile_pool(name="ew", bufs=4))
        dump_pool = ctx.enter_context(tc.tile_pool(name="dmp", bufs=3))
        k_pool = ctx.enter_context(tc.tile_pool(name="kp", bufs=2))
        q_pool = ctx.enter_context(tc.tile_pool(name="qp", bufs=2))
        kscr_pool = ctx.enter_context(tc.tile_pool(name="ks", bufs=2))

        # ---- resident inputs (priority order for DMA) --------------------
        qlhs = const.tile([128, KC, NL], qdt, tag="qlhs")
        nc.sync.dma_start(qlhs[:, :, :], qlhs_d[:, :, :])
        rhs0 = const.tile([128, KC, TW], qdt, tag="rhs0")
        nc.sync.dma_start(rhs0[:, :, :], qrhs_d[:, :, 0:TW])
        if mode == "fp8dr":
            lwdr = const.tile([1, 2, N], FP8, tag="lwdr")
            nc.sync.dma_start(lwdr[:, :, :], lwdr_d[:, :, :])
        else:
            lwrow = const.tile([1, N], F16, tag="lwrow")
            nc.sync.dma_start(lwrow[:, :], lwrow_d[:, :])
        colid = const.tile([128, 128], F16, tag="colid")
        nc.sync.dma_start(colid[:, :], colid_d[:, :])
        pidx = const.tile([128, 1], F32, tag="pidx")
        nc.sync.dma_start(pidx[:, :], pidx_d[:, :])
        yrow = const.tile([128, NB], F32, tag="yrow")
        nc.sync.dma_start(yrow[:, :], yrow_d[:, :])
        crow = const.tile([128, NB], F32, tag="crow")
        nc.sync.dma_start(crow[:, :], crow_d[:, :])
        dinv = const.tile([128, NB], F32, tag="dinv")
        nc.sync.dma_start(dinv[:, :], dinv_d[:, :])
        ybc = const.tile([128, N], F16, tag="ybc")
        nc.sync.dma_start(ybc[:, :], ybc_d[:, :])

        if mode == "fp8dr":
            ones_k2 = const.tile([1, 2, 128], FP8, tag="ones_k2")
            nc.vector.memset(ones_k2[:, :, :], 1.0)
        else:
            ones_k1 = const.tile([1, 128], F16, tag="ones_k1")
            nc.vector.memset(ones_k1[:, :], 1.0)
        ones_col = const.tile([128, 1], F32, tag="ones_col")
        nc.vector.memset(ones_col[:, :], 1.0)

        # accumulator slots
        aslt = const.tile([128, NB * NS2], F32, tag="aslt")
        bslt = const.tile([128, NB * NS2], F32, tag="bslt")
        kss = const.tile([128, NB * KP], F32, tag="kss")
        kpos = const.tile([128, NB], F32, tag="kpos")

        kpb = (NB + NS - 1) // NS  # k-path blocks per column tile

        # ---- main loop: score slab ---------------------------------------
        for s in range(NS):
            if s == 0:
                rhs = rhs0
            else:
                rhs = rh_pool.tile([128, KC, TW], qdt, tag="rh", name=f"rhs{s}")
                nc.sync.dma_start(rhs[:, :, :], qrhs_d[:, :, s * TW:(s + 1) * TW])

            # interleave k-path blocks with the main loop
            for j in range(kpb):
                bk = s * kpb + j
                if bk >= NB:
                    continue
                kt = k_pool.tile([128, KP * D], F16, tag="kt")
                nc.sync.dma_start(kt[:, :], kr_d[bk, :, :])
                qt = q_pool.tile([128, D], F16, tag="qt")
                nc.sync.dma_start(qt[:, :], qr_d[bk, :, :])
                for kk in range(KP):
                    kscr = kscr_pool.tile([128, D], F16, tag="kscr")
                    ke.scalar_tensor_tensor(
                        kscr[:, :], kt[:, kk * D:(kk + 1) * D], 1.0,
                        qt[:, :],
                        op0=mybir.AluOpType.mult, op1=mybir.AluOpType.mult,
                        accum_out=kss[:, bk * KP + kk: bk * KP + kk + 1])
                ksse = const.tile([128, KP], F32, tag=f"ksse{bk}")
                nc.scalar.activation(
                    ksse[:, :],
                    kss[:, bk * KP:(bk + 1) * KP],
                    mybir.ActivationFunctionType.Exp, scale=float(1.0 / TAU),
                    accum_out=kpos[:, bk:bk + 1])

            for b in range(NB):
                for h in range(2):
                    hc0 = h * TWH                 # column offset within tile
                    ps = psum_pool.tile([128, TWH], F32, tag="ps")
                    # c-outer order: each weight load feeds all NCH chunks
                    if mode == "fp8dr":
                        for c2 in range(KC // 2):
                            for nch in range(NCH):
                                r0 = hc0 + nch * 512
                                nc.tensor.matmul(
                                    ps[:, nch * 512:(nch + 1) * 512],
                                    qlhs[:, 2 * c2:2 * c2 + 2, b * 128:(b + 1) * 128],
                                    rhs[:, 2 * c2:2 * c2 + 2, r0:r0 + 512],
                                    start=(c2 == 0), stop=False,
                                    perf_mode=mybir.MatmulPerfMode.DoubleRow)
                        for nch in range(NCH):
                            r0 = hc0 + nch * 512
                            nc.tensor.matmul(
                                ps[:, nch * 512:(nch + 1) * 512],
                                ones_k2[0:1, :, :],
                                lwdr[0:1, :, s * TW + r0: s * TW + r0 + 512],
                                start=False, stop=True,
                                perf_mode=mybir.MatmulPerfMode.DoubleRow)
                    else:
                        for c in range(KC):
                            for nch in range(NCH):
                                r0 = hc0 + nch * 512
                                nc.tensor.matmul(
                                    ps[:, nch * 512:(nch + 1) * 512],
                                    qlhs[:, c, b * 128:(b + 1) * 128],
                                    rhs[:, c, r0:r0 + 512],
                                    start=(c == 0), stop=False)
                        for nch in range(NCH):
                            r0 = hc0 + nch * 512
                            nc.tensor.matmul(
                                ps[:, nch * 512:(nch + 1) * 512],
                                ones_k1[0:1, :],
                                lwrow[0:1, s * TW + r0: s * TW + r0 + 512],
                                start=False, stop=True)
                    hd = (b * 128) // TWH
                    if s == 0 and h == hd:
                        # zero the 128 diagonal scores of this row block
                        dc = b * 128 - hd * TWH
                        nc.vector.scalar_tensor_tensor(
                            ps[:, dc:dc + 128],
                            colid[:, :], pidx[:, 0:1],
                            ps[:, dc:dc + 128],
                            op0=mybir.AluOpType.not_equal,
                            op1=mybir.AluOpType.mult)
                    slot = b * NS2 + 2 * s + h
                    ew = ew_pool.tile([128, TWH], F32)
                    nc.scalar.activation(ew[:, :], ps[:, :],
                                         mybir.ActivationFunctionType.Exp,
                                         scale=exp_scale,
                                         accum_out=aslt[:, slot:slot + 1])
                    # B: same-class weighted row-sum
                    d2 = dump_pool.tile([128, TWH], F16)
                    nc.vector.scalar_tensor_tensor(
                        d2[:, :],
                        ybc[:, s * TW + hc0: s * TW + hc0 + TWH],
                        yrow[:, b:b + 1],
                        ew[:, :],
                        op0=mybir.AluOpType.is_equal, op1=mybir.AluOpType.mult,
                        accum_out=bslt[:, slot:slot + 1])

        # ---- finalize (vectorized over [128, NB]) ------------------------
        acolM = const.tile([128, NB], F32, tag="acolM")
        bcolM = const.tile([128, NB], F32, tag="bcolM")
        for b in range(NB):
            nc.vector.tensor_reduce(acolM[:, b:b + 1], aslt[:, b * NS2:(b + 1) * NS2],
                                    mybir.AxisListType.X, mybir.AluOpType.add)
            nc.vector.tensor_reduce(bcolM[:, b:b + 1], bslt[:, b * NS2:(b + 1) * NS2],
                                    mybir.AxisListType.X, mybir.AluOpType.add)
        bm1 = const.tile([128, NB], F32, tag="bm1")
        nc.vector.tensor_scalar_add(bm1[:, :], bcolM[:, :], -1.0)
        numM = const.tile([128, NB], F32, tag="numM")
        # numM = kpos + crow * (B - 1)
        nc.vector.scalar_tensor_tensor(
            numM[:, :], bm1[:, :], 1.0, crow[:, :],
            op0=mybir.AluOpType.mult, op1=mybir.AluOpType.mult)
        nc.vector.tensor_add(numM[:, :], numM[:, :], kpos[:, :])
        denM = const.tile([128, NB], F32, tag="denM")
        nc.vector.tensor_sub(denM[:, :], acolM[:, :], bcolM[:, :])
        denL = const.tile([128, NB], F32, tag="denL")
        nc.scalar.activation(denL[:, :], denM[:, :], mybir.ActivationFunctionType.Ln)
        numL = const.tile([128, NB], F32, tag="numL")
        nc.scalar.activation(numL[:, :], numM[:, :], mybir.ActivationFunctionType.Ln)
        diffM = const.tile([128, NB], F32, tag="diffM")
        nc.vector.tensor_sub(diffM[:, :], denL[:, :], numL[:, :])
        losscol = const.tile([128, NB], F32, tag="losscol")
        nc.vector.tensor_mul(losscol[:, :], diffM[:, :], dinv[:, :])

        # ---- reduce to a single partial ----------------------------------
        lsum = const.tile([128, 1], F32, tag="lsum")
        nc.vector.tensor_reduce(lsum[:, :], losscol[:, :],
                                mybir.AxisListType.X, mybir.AluOpType.add)
        psf = psum_pool.tile([128, TWH], F32, tag="ps")
        nc.tensor.matmul(psf[0:1, 0:1], lsum[:, :],
                         ones_col[:, :], start=True, stop=True)
        outsb = const.tile([1, 1], F32, tag="outsb")
        nc.scalar.copy(outsb[0:1, 0:1], psf[0:1, 0:1])
        nc.sync.dma_start(out_d[:, :], outsb[0:1, 0:1])

    nc.compile()
    return nc


# ---------------------------------------------------------------------------
# host-side marshalling
# ---------------------------------------------------------------------------

def make_inputs(q, k, y, cfg: Cfg):
    """Build the per-core input maps (layout/replication marshalling)."""
    N, D, KP, TW = cfg.N, cfg.D, cfg.KP, cfg.TW
    NL, NB, NS, KC = cfg.NL, cfg.NB, cfg.NS, cfg.KC
    q = np.asarray(q, dtype=np.float32)
    k = np.asarray(k, dtype=np.float32)
    y = np.asarray(y).astype(np.int64)

    qdt_np = {"fp8dr": ml_dtypes.float8_e4m3fn,
              "bf16": ml_dtypes.bfloat16,
              "f32r": np.float32}[cfg.mode]
    qs = (q * cfg.prescale).astype(qdt_np)          # [N, D] quantized

    counts = np.bincount(y, minlength=NUM_CLASSES)
    w16 = (1.0 / counts[y].astype(np.float64)).astype(np.float16)  # [N]
    # lw = ln(w) * prescale^2 * TAU, so exp(scale * psum) folds in w exactly
    lw = (np.log(w16.astype(np.float64))
          * cfg.prescale * cfg.prescale * TAU)
    lw16 = lw.astype(np.float16)
    lw8_hi = lw.astype(ml_dtypes.float8_e4m3fn)
    lw8_lo = (lw - lw8_hi.astype(np.float64)).astype(ml_dtypes.float8_e4m3fn)
    ylab = (y + 1).astype(np.float16)                              # labels 1..C

    colid = np.broadcast_to(np.arange(128, dtype=np.float16)[None, :],
                            (128, 128)).copy()
    pidx = np.arange(128, dtype=np.float32).reshape(128, 1).copy()

    in_maps = []
    for r in range(NCORES):
        rows = slice(r * NL, (r + 1) * NL)
        rot = (np.arange(N) + r * NL) % N

        # lhsT chunks: qlhs[p, c, i] = qs[r*NL+i, c*128+p]
        qlhs = np.ascontiguousarray(
            qs[rows].T.reshape(KC, 128, NL).transpose(1, 0, 2))
        # rhs chunks, rotated: qrhs[p, c, j] = qs[rot(j), c*128+p]
        qrhs = np.ascontiguousarray(
            qs[rot].T.reshape(KC, 128, N).transpose(1, 0, 2))

        ybc = np.broadcast_to(ylab[rot][None, :], (128, N)).copy()

        yloc = y[rows]
        yrowm = np.ascontiguousarray(
            (yloc + 1).astype(np.float32).reshape(NB, 128).T)
        crowm = np.ascontiguousarray(
            counts[yloc].astype(np.float32).reshape(NB, 128).T)
        dinvm = np.ascontiguousarray(
            (1.0 / (counts[yloc] - 1 + KP)).astype(np.float32).reshape(NB, 128).T)

        kr = np.ascontiguousarray(
            k[rows].reshape(NB, 128, KP * D)).astype(np.float16)
        qr = np.ascontiguousarray(
            q[rows].reshape(NB, 128, D)).astype(np.float16)

        imap = {
            "qlhs": qlhs, "qrhs": qrhs,
            "ybc": ybc, "colid": colid, "pidx": pidx,
            "yrow": yrowm, "crow": crowm, "dinv": dinvm,
            "kr": kr, "qr": qr,
        }
        if cfg.mode == "fp8dr":
            imap["lwdr"] = np.stack([lw8_hi[rot], lw8_lo[rot]]).reshape(1, 2, N)
        else:
            imap["lwrow"] = lw16[rot].reshape(1, N).copy()
        in_maps.append(imap)
    return in_maps


_CACHE = {}


def _get_nc(mode="bf16"):
    if mode not in _CACHE:
        cfg = Cfg(mode=mode)
        _CACHE[mode] = (cfg, build_bass(cfg))
    return _CACHE[mode]


def kernel(q, k, y, trace=False, mode="bf16"):
    cfg, nc = _get_nc(mode)
    in_maps = make_inputs(q, k, y, cfg)
    res = run_bass_kernel_spmd(nc, in_maps, core_ids=list(range(NCORES)),
                               trace=trace)
    total = np.sum([res.results[r]["out"][0, 0] for r in range(NCORES)],
                   dtype=np.float64)
    out = np.asarray(total / cfg.N, dtype=np.float32)
    if trace:
        kernel.last_results = res
    return out
